# revision 5
# baseline (speedup 1.0000x reference)
"""BERT per-word mean-pool (segment reduce) on 8 Trainium2 NeuronCores.

Problem: output[B=64, S=512, E=768] f32, mappings[B, W=255] int32 (values 1 or 2).
Per sentence, strip [CLS]/[SEP], mean-pool contiguous BPE spans into word vectors.

Key identity: every word's span has 1 or 2 BPE rows.  With s = span start,
    out[w] = (1/m) * (hs rows s .. s+m-1 summed),  m in {1, 2}.

Sharding: pure data parallel, 8 sentences per core, no cross-core comms.
All device data in bf16 (tolerance 2e-2 >> bf16's ~2e-3); host casts.

Two competitive kernels, HW-measured:

* mode "mm" (default): segment-sum as matmul. NO gathers, NO Q7 ucode --
  dodges the ~16us ucode-ready gate that floors every dma_gather kernel.
  Contiguous loads of x (SWDGE+HWDGE alternating); the 0/1 segment matrix
  M^T[t, w] = [s_w <= t] - [s_{w+1} <= t] is built on-chip from an uploaded
  boundary row (2 DVE ops per 128-row k-tile); TensorE accumulates
  out = M^T.T @ x per (sentence, 128-word tile) in PSUM; ACT evicts with a
  per-partition 1/m scale; HWDGE stores.

* mode "wh": windowed ucode gather. One 2-row-window descriptor per word
  (InstDMAGatherAnt), x pre-scaled by 1/m on host, single fused DVE
  scalar_tensor_tensor per word-column: out = t1*[m==2] + t0.

Raw indirect InstDMACopy (modes ind/acc/wind) is broken on this runtime:
the DGE consumes ONE offset per partition of the out AP and fetches
consecutive rows after it (HW-probed); a 3D out AP makes it worse. Do not
use those modes.
"""

import numpy as np

from concourse import bacc, bass, mybir, tile
from concourse.bass_utils import run_bass_kernel_spmd

B, S, W, E = 64, 512, 255, 768
NCORES = 8
BPC = B // NCORES            # sentences per core
NW = BPC * W                 # 2040 real words per core
NWP = 2048                   # padded word count (multiple of 512)
NCHUNK = 4                   # chunks per core
CPW = NWP // NCHUNK          # 512 words per chunk
JJ = CPW // 128              # 4 words per partition per chunk
ROWS = BPC * S               # 4096 input rows per core
NIDX = 2 * CPW               # 1024 gather indices per chunk (A then B)

_F32 = mybir.dt.float32
_BF16 = mybir.dt.bfloat16
_I16 = mybir.dt.int16

_NPBF16 = None


def _np_bf16():
    global _NPBF16
    if _NPBF16 is None:
        import ml_dtypes

        _NPBF16 = ml_dtypes.bfloat16
    return _NPBF16


def _build_nc(reps=1, bufs=2, order="pc", nq=1, mode="ab", nchunk=NCHUNK,
              merged_idx=False, warm=False, hscale=False, sizes=None,
              dt="f32", split0=False):
    _DT = _F32 if dt == "f32" else _BF16
    if sizes is not None:
        return _build_nc_sized(reps, bufs, sizes, hscale)
    if mode == "ind":
        return _build_nc_ind(reps, bufs, dt, nchunk, hscale)
    if mode == "acc":
        return _build_nc_acc(reps, bufs, dt, nchunk)
    if mode == "wind":
        return _build_nc_wind(reps, bufs, dt, nchunk)
    if mode == "mm":
        return _build_nc_mm(reps)
    nc = bacc.Bacc(
        "TRN2",
        target_bir_lowering=False,
        debug=False,
        num_devices=NCORES,
        num_swdge_queues=nq,
    )
    x = nc.dram_tensor("x", [ROWS, E], _DT, kind="ExternalInput").ap()
    # indices are int16, wrapped [i%16, i//16] into 16 partitions and
    # replicated 8x down to 128 partitions (Q7 core replication).
    cpw = NWP // nchunk
    jj = cpw // 128
    nidx = 2 * cpw if mode == "ab" else cpw
    idx = nc.dram_tensor(
        "idx", [nchunk, 128, nidx // 16], _I16, kind="ExternalInput"
    ).ap()
    _RDT = _F32 if mode == "ws" else _DT  # tensor_scalar wants f32 scalars
    if mode in ("win", "ws"):
        # per word w: rw[p, 2c] = 1/m(w), rw[p, 2c+1] = (m(w)-1)/m(w)
        rw = nc.dram_tensor(
            "rw", [nchunk, 128, 2 * jj], _RDT, kind="ExternalInput"
        ).ap()
    if mode == "wh":
        # per word w: bt[p, c] = [m(w) == 2]; x rows pre-scaled by 1/m on host
        bt = nc.dram_tensor(
            "bt", [nchunk, 128, jj], _DT, kind="ExternalInput"
        ).ap()
    y = nc.dram_tensor("y", [NWP, E], _DT, kind="ExternalOutput").ap()

    with tile.TileContext(nc) as tc:
        with (
            tc.tile_pool(name="idxp", bufs=1) as ipool,
            tc.tile_pool(name="io", bufs=bufs) as pool,
        ):
            if warm:
                # dummy 16-index gather issued first: triggers the Q7
                # ucode IRAM fetch (~6us) while the idx loads stream in,
                # so the first real gather isn't stalled on it.
                wi = ipool.tile([128, 1], _I16, tag="warmi")
                nc.gpsimd.memset(wi[:], 0)
                wo = ipool.tile([128, E], _DT, tag="warmo")
                nc.gpsimd.dma_gather(
                    wo[:].rearrange("p (c e) -> p c e", e=E),
                    x[:, :], wi[:], 16, 16, E,
                )
            its, rts = [], []
            ncols = nidx // 16
            if merged_idx:
                its = []
                if split0:
                    # chunk-0 idx as its own tiny first DMA so the first
                    # gather isn't gated on the full idx upload
                    it0 = ipool.tile([128, ncols], _I16, tag="it0")
                    nc.sync.dma_start(out=it0[:], in_=idx[0])
                    its.append(it0[:])
                    itall = ipool.tile(
                        [128, (nchunk - 1) * ncols], _I16, tag="itall"
                    )
                    nc.sync.dma_start(
                        out=itall[:].rearrange(
                            "p (q s) -> p q s", q=nchunk - 1
                        ),
                        in_=idx[1:].rearrange("q p s -> p q s"),
                    )
                    its += [
                        itall[:, q * ncols : (q + 1) * ncols]
                        for q in range(nchunk - 1)
                    ]
                else:
                    itall = ipool.tile([128, nchunk * ncols], _I16, tag="itall")
                    nc.sync.dma_start(
                        out=itall[:].rearrange("p (q s) -> p q s", q=nchunk),
                        in_=idx.rearrange("q p s -> p q s"),
                    )
                    its = [
                        itall[:, q * ncols : (q + 1) * ncols]
                        for q in range(nchunk)
                    ]
            else:
                for q in range(nchunk):
                    it = ipool.tile([128, ncols], _I16, tag=f"it{q}")
                    nc.sync.dma_start(out=it[:], in_=idx[q])
                    its.append(it[:])
            if mode in ("win", "ws"):
                # single merged weight load (one HWDGE DMA for all chunks)
                rtall = ipool.tile([128, nchunk * 2 * jj], _RDT, tag="rtall")
                nc.sync.dma_start(
                    out=rtall[:].rearrange("p (q s) -> p q s", q=nchunk),
                    in_=rw.rearrange("q p s -> p q s"),
                )
                rts = [
                    rtall[:, q * 2 * jj : (q + 1) * 2 * jj]
                    for q in range(nchunk)
                ]
            if mode == "wh":
                btall = ipool.tile([128, nchunk * jj], _DT, tag="btall")
                nc.sync.dma_start(
                    out=btall[:].rearrange("p (q s) -> p q s", q=nchunk),
                    in_=bt.rearrange("q p s -> p q s"),
                )
            for _rep in range(reps):
                for q in range(nchunk):
                    if mode == "ab":
                        # gathered slot i -> T[i % 128, i // 128, :]
                        # i = c*128 + p:  c in 0..3 -> first-BPE row of word
                        # w = q*512 + p*4 + c;  c in 4..7 -> last-BPE row.
                        t = pool.tile([128, 2 * jj * E], _DT, tag="t")
                        nc.gpsimd.dma_gather(
                            t[:].rearrange("p (c e) -> p c e", e=E),
                            x[:, :],
                            its[q],
                            nidx,
                            nidx,
                            E,
                            queue_num=q % nq,
                        )
                        c = pool.tile([128, jj * E], _DT, tag="c")
                        nc.vector.tensor_add(
                            out=c[:], in0=t[:, : jj * E], in1=t[:, jj * E :]
                        )
                        if not hscale:
                            nc.scalar.mul(c[:], c[:], 0.5)
                    else:
                        # one 2-row window [s, s+2) per word, 6KB descriptors;
                        # out[w] = win[0]*r1 + win[1]*r2 kills the junk row
                        # (m=1: r=(1,0); m=2: r=(.5,.5)).
                        t = pool.tile([128, 2 * jj * E], _DT, tag="t")
                        xw = bass.AP(x.tensor, 0, [[E, ROWS - 1], [1, 2 * E]])
                        nc.gpsimd.dma_gather(
                            t[:].rearrange("p (c e) -> p c e", e=2 * E),
                            xw,
                            its[q],
                            cpw,
                            cpw,
                            2 * E,
                            elem_step=E,
                            queue_num=q % nq,
                        )
                        c = pool.tile([128, jj * E], _DT, tag="c")
                        if mode == "wh":
                            # x rows pre-scaled by 1/m on host; one fused DVE
                            # op per word-column: out = t1*[m==2] + t0, all
                            # APs contiguous [128, E] (full-rate DVE).
                            for j in range(jj):
                                nc.vector.scalar_tensor_tensor(
                                    out=c[:, j * E : (j + 1) * E],
                                    in0=t[:, (2 * j + 1) * E : (2 * j + 2) * E],
                                    scalar=btall[:, q * jj + j : q * jj + j + 1],
                                    in1=t[:, 2 * j * E : (2 * j + 1) * E],
                                    op0=mybir.AluOpType.mult,
                                    op1=mybir.AluOpType.add,
                                )
                        elif mode == "ws":
                            # device weights, contiguous slices: per word-col
                            # c3 = t1*r2; c = t0*r1 + c3
                            c3 = pool.tile([128, jj * E], _DT, tag="c3")
                            for j in range(jj):
                                nc.vector.tensor_scalar_mul(
                                    c3[:, j * E : (j + 1) * E],
                                    t[:, (2 * j + 1) * E : (2 * j + 2) * E],
                                    rts[q][:, 2 * j + 1 : 2 * j + 2],
                                )
                                nc.vector.scalar_tensor_tensor(
                                    out=c[:, j * E : (j + 1) * E],
                                    in0=t[:, 2 * j * E : (2 * j + 1) * E],
                                    scalar=rts[q][:, 2 * j : 2 * j + 1],
                                    in1=c3[:, j * E : (j + 1) * E],
                                    op0=mybir.AluOpType.mult,
                                    op1=mybir.AluOpType.add,
                                )
                        else:
                            t3 = t[:].rearrange("p (c e) -> p c e", e=2 * E)
                            r3 = rts[q].rearrange("p (c f) -> p c f", f=2)
                            c3 = pool.tile([128, jj * E], _DT, tag="c3")
                            cv = c[:].rearrange("p (j e) -> p j e", e=E)
                            c3v = c3[:].rearrange("p (j e) -> p j e", e=E)
                            nc.vector.tensor_tensor(
                                out=cv,
                                in0=t3[:, :, :E],
                                in1=r3[:, :, 0:1].to_broadcast([128, jj, E]),
                                op=mybir.AluOpType.mult,
                            )
                            nc.vector.tensor_tensor(
                                out=c3v,
                                in0=t3[:, :, E:],
                                in1=r3[:, :, 1:2].to_broadcast([128, jj, E]),
                                op=mybir.AluOpType.mult,
                            )
                            nc.vector.tensor_add(out=c[:], in0=c[:], in1=c3[:])
                    ychunk = y[q * cpw : (q + 1) * cpw, :]
                    if order == "pc":
                        nc.sync.dma_start(
                            out=ychunk.rearrange("(p j) e -> p (j e)", p=128),
                            in_=c[:],
                        )
                    else:
                        nc.sync.dma_start(
                            out=ychunk.rearrange("(j p) e -> p j e", p=128),
                            in_=c[:].rearrange("p (j e) -> p j e", e=E),
                        )
    nc.compile()
    return nc


def _build_nc_ind(reps, bufs, dt="f32", nchunk=4, hscale=False, tbufs=1):
    """Native SWDGE indirect gather (no ucode library -> no ~16us Q7 ucode
    ready gate). One InstDMACopy per chunk reading a column slice of a single
    int32 offset tile; slices advance monotonically (the HW-verified-safe
    pattern). Per chunk: first wpp cols = first-BPE rows, next wpp cols =
    last-BPE rows of words w = q*cpw + p*wpp + j."""
    _DT = _F32 if dt == "f32" else _BF16
    cpw = NWP // nchunk
    wpp = cpw // 128              # words per partition per chunk
    nc = bacc.Bacc(
        "TRN2", target_bir_lowering=False, debug=False, num_devices=NCORES
    )
    x = nc.dram_tensor("x", [ROWS, E], _DT, kind="ExternalInput").ap()
    idx = nc.dram_tensor("idx", [128, 2 * wpp * nchunk], mybir.dt.int32,
                         kind="ExternalInput").ap()
    y = nc.dram_tensor("y", [NWP, E], _DT, kind="ExternalOutput").ap()
    with tile.TileContext(nc) as tc:
        with (
            tc.tile_pool(name="idxp", bufs=1) as ipool,
            tc.tile_pool(name="tp", bufs=tbufs) as tpool,
            tc.tile_pool(name="io", bufs=bufs) as pool,
        ):
            itall = ipool.tile([128, 2 * wpp * nchunk], mybir.dt.int32,
                               tag="itall")
            nc.sync.dma_start(out=itall[:], in_=idx[:, :])
            for _rep in range(reps):
                for q in range(nchunk):
                    # tbufs=1 pool: WAR dep guarantees at most one indirect
                    # DMA in flight (two concurrent ones corrupt offsets).
                    t = tpool.tile([128, 2 * wpp * E], _DT, tag="t")
                    nc.gpsimd.indirect_dma_start(
                        # 3D out AP: one offset consumed per (p, c) row slot.
                        # A flat [128, 2*wpp*E] out makes the DGE take ONE
                        # offset per partition and fetch consecutive rows
                        # (HW-probed failure mode).
                        out=t[:].rearrange("p (c e) -> p c e", e=E),
                        out_offset=None,
                        in_=x[:, :],
                        in_offset=bass.IndirectOffsetOnAxis(
                            ap=itall[:, 2 * wpp * q : 2 * wpp * (q + 1)],
                            axis=0,
                        ),
                    )
                    c = pool.tile([128, wpp * E], _DT, tag="c")
                    nc.vector.tensor_add(
                        out=c[:], in0=t[:, : wpp * E], in1=t[:, wpp * E :]
                    )
                    if not hscale:
                        nc.scalar.mul(c[:], c[:], 0.5)
                    nc.sync.dma_start(
                        out=y[q * cpw : (q + 1) * cpw, :].rearrange(
                            "(p j) e -> p (j e)", p=128
                        ),
                        in_=c[:],
                    )
    nc.compile()
    return nc


def _build_nc_wind(reps, bufs, dt, nchunk, tbufs=1):
    """Native indirect gather of 2-row windows (one InstDMACopy per chunk,
    offsets in raw elements via a 1-D source view; coef=1), then the wh-style
    fused STT combine (x host-scaled by 1/m, beta kills junk rows)."""
    _DT = _F32 if dt == "f32" else _BF16
    cpw = NWP // nchunk
    wpp = cpw // 128
    nc = bacc.Bacc(
        "TRN2", target_bir_lowering=False, debug=False, num_devices=NCORES
    )
    x = nc.dram_tensor("x", [ROWS, E], _DT, kind="ExternalInput").ap()
    idx = nc.dram_tensor("idx", [128, wpp * nchunk], mybir.dt.int32,
                         kind="ExternalInput").ap()
    bt = nc.dram_tensor("bt", [nchunk, 128, wpp], _DT,
                        kind="ExternalInput").ap()
    y = nc.dram_tensor("y", [NWP, E], _DT, kind="ExternalOutput").ap()
    with tile.TileContext(nc) as tc:
        with (
            tc.tile_pool(name="idxp", bufs=1) as ipool,
            tc.tile_pool(name="tp", bufs=tbufs) as tpool,
            tc.tile_pool(name="io", bufs=bufs) as pool,
        ):
            itall = ipool.tile([128, wpp * nchunk], mybir.dt.int32,
                               tag="itall")
            nc.sync.dma_start(out=itall[:], in_=idx[:, :])
            btall = ipool.tile([128, nchunk * wpp], _DT, tag="btall")
            nc.sync.dma_start(
                out=btall[:].rearrange("p (q s) -> p q s", q=nchunk),
                in_=bt.rearrange("q p s -> p q s"),
            )
            for _rep in range(reps):
                for q in range(nchunk):
                    t = tpool.tile([128, 2 * wpp * E], _DT, tag="t")
                    # coef = E (row stride); each (p, c) out slot pulls 2E
                    # contiguous elements = the 2-row window [s, s+2)
                    nc.gpsimd.indirect_dma_start(
                        out=t[:].rearrange("p (c e) -> p c e", e=2 * E),
                        out_offset=None,
                        in_=x[:, :],
                        in_offset=bass.IndirectOffsetOnAxis(
                            ap=itall[:, wpp * q : wpp * (q + 1)], axis=0),
                    )
                    c = pool.tile([128, wpp * E], _DT, tag="c")
                    for j in range(wpp):
                        nc.vector.scalar_tensor_tensor(
                            out=c[:, j * E : (j + 1) * E],
                            in0=t[:, (2 * j + 1) * E : (2 * j + 2) * E],
                            scalar=btall[:, q * wpp + j : q * wpp + j + 1],
                            in1=t[:, 2 * j * E : (2 * j + 1) * E],
                            op0=mybir.AluOpType.mult,
                            op1=mybir.AluOpType.add,
                        )
                    nc.sync.dma_start(
                        out=y[q * cpw : (q + 1) * cpw, :].rearrange(
                            "(p j) e -> p (j e)", p=128),
                        in_=c[:],
                    )
    nc.compile()
    return nc


def _build_nc_mm(reps=1, xbufs=1, pbufs=4, obufs=6):
    """Segment-sum as matmul: NO gathers, NO Q7 ucode (dodges the ~16us
    ucode-ready gate). Contiguous HWDGE loads of x; the 0/1 segment matrix
    M^T[t, w] = [s_w <= t] - [s_{w+1} <= t] is built on-chip from an
    uploaded (replicated) boundary row via two DVE ops per k-tile; TensorE
    computes out = M^T.T @ x per (sentence, word-tile); PSUM is evicted with
    a per-partition 1/m scale (the span mean) straight to bf16 and stored.

    Word layout: y row = sent*256 + w (pad word 255 per sentence interleaved).
    """
    NS = BPC                      # sentences per core = 8
    KT = S // 128                 # k-tiles per sentence = 4
    WT = 2                        # word-tiles per sentence (256 words)
    SW = 257                      # boundary cols per sentence (s_0..s_255, sentinel)
    nc = bacc.Bacc(
        "TRN2", target_bir_lowering=False, debug=False, num_devices=NCORES
    )
    x = nc.dram_tensor("x", [ROWS, E], _BF16, kind="ExternalInput").ap()
    sth = nc.dram_tensor("sth", [1, NS * SW], mybir.dt.float16,
                         kind="ExternalInput").ap()
    gcol = nc.dram_tensor("gcol", [128, KT], _F32, kind="ExternalInput").ap()
    msc = nc.dram_tensor("msc", [128, NS * WT], _F32,
                         kind="ExternalInput").ap()
    y = nc.dram_tensor("y", [NS * 256, E], _BF16, kind="ExternalOutput").ap()

    with tile.TileContext(nc) as tc:
        with (
            tc.tile_pool(name="const", bufs=1) as cpool,
            tc.tile_pool(name="xs", bufs=xbufs) as xpool,
            tc.tile_pool(name="m", bufs=1) as mpool,
            tc.tile_pool(name="ps", bufs=pbufs, space="PSUM") as ppool,
            tc.tile_pool(name="out", bufs=obufs) as opool,
        ):
            # x loads split across the SWDGE ring (gpsimd) and the ACT
            # engine's own HWDGE ring (qActDynamicHW) -- two descriptor
            # paths inject in parallel; the sync ring stays FIFO-clean for
            # the stores (mixing big loads with stores on one ring
            # head-of-line-blocks them; HW-measured +14us)
            # boundary row: 4KB flat upload + PE broadcast to 128 partitions
            # (a [128, 2056] replicated upload costs 0.53MB of HBM stream;
            # ones-matmul replication costs ~0)
            sfl = cpool.tile([1, NS * SW], mybir.dt.float16, tag="sfl")
            nc.sync.dma_start(out=sfl[:], in_=sth[:, :])
            ones = cpool.tile([1, 128], mybir.dt.float16, tag="ones")
            nc.vector.memset(ones[:], 1.0)
            stht = cpool.tile([128, NS * SW], mybir.dt.float16, tag="sth")
            off = 0
            while off < NS * SW:
                n = min(384, NS * SW - off)
                pb = ppool.tile([128, 384], _F32, tag="p0")
                nc.tensor.matmul(
                    pb[:, :n], ones[:], sfl[:, off : off + n],
                    start=True, stop=True,
                )
                nc.vector.tensor_copy(out=stht[:, off : off + n], in_=pb[:, :n])
                off += n
            gct = cpool.tile([128, KT], _F32, tag="gcol")
            nc.sync.dma_start(out=gct[:], in_=gcol[:, :])
            msct = cpool.tile([128, NS * WT], _F32, tag="msc")
            nc.sync.dma_start(out=msct[:], in_=msc[:, :])
            # last sentence per ring (s6 scalar, s7 gpsimd) split (k0-2)+(k3):
            # its wt0 matmuls overlap the final k3 transfer, shortening the
            # post-last-load chain
            xts = []
            xbs = {}
            for s in range(NS):
                eng = nc.scalar if s % 2 == 0 else nc.gpsimd
                xin = x[s * S : (s + 1) * S, :].rearrange(
                    "(k p) e -> p k e", p=128
                )
                if s >= NS - 2:
                    xa = xpool.tile([128, 3 * E], _BF16, tag=f"xa{s}")
                    eng.dma_start(
                        out=xa[:].rearrange("p (k e) -> p k e", e=E),
                        in_=x[s * S : s * S + 384, :].rearrange(
                            "(k p) e -> p k e", p=128
                        ),
                    )
                    xb = xpool.tile([128, E], _BF16, tag=f"xb{s}")
                    eng.dma_start(out=xb[:], in_=x[s * S + 384 : (s + 1) * S, :])
                    xts.append(xa)
                    xbs[s] = xb
                else:
                    xt = xpool.tile([128, KT * E], _BF16, tag=f"x{s}")
                    eng.dma_start(
                        out=xt[:].rearrange("p (k e) -> p k e", e=E),
                        in_=xin,
                    )
                    xts.append(xt)
            for _rep in range(reps):
                # cmp_k[p, sent*SW + w] = [s_w <= p + 128k]  (0/1 bf16)
                # build order: each cmp immediately followed by its s0 sub,
                # so (s0, wt0)'s blocks are ready ~1us sooner and the first
                # matmul (hence the whole eviction chain) starts earlier
                cmps = []
                mts = {}

                def _sub(k, s):
                    mt = mpool.tile([128, 256], _BF16, tag=f"m{k}_{s}")
                    nc.vector.tensor_tensor(
                        out=mt[:],
                        in0=cmps[k][:, s * SW : s * SW + 256],
                        in1=cmps[k][:, s * SW + 1 : s * SW + 257],
                        op=mybir.AluOpType.subtract,
                    )
                    mts[(k, s)] = mt

                for k in range(KT):
                    ck = mpool.tile([128, NS * SW], _BF16, tag=f"cmp{k}")
                    nc.vector.tensor_scalar(
                        ck[:], stht[:], gct[:, k : k + 1], None,
                        op0=mybir.AluOpType.is_le,
                    )
                    cmps.append(ck)
                    _sub(k, 0)
                for s in range(1, NS):
                    for k in range(KT):
                        _sub(k, s)
                # per (sent, wt): 3 k-blocks x 2 N-halves accumulate in PSUM
                for s in range(NS):
                    # one merged [128, 2E] out tile per sentence -> ONE store
                    # (8 stores instead of 16: half the issue/receipt cost)
                    ot = opool.tile([128, 2 * E], _BF16, tag="o")
                    for wt in range(WT):
                        ks = (0, 1, 2) if wt == 0 else (1, 2, 3)
                        pts = []
                        for h in range(2):
                            pt = ppool.tile([128, 384], _F32, tag=f"p{h}")
                            for i, k in enumerate(ks):
                                if k == 3 and s in xbs:
                                    rhs = xbs[s][:, h * 384 : (h + 1) * 384]
                                else:
                                    rhs = xts[s][:, k * E + h * 384 :
                                                 k * E + (h + 1) * 384]
                                nc.tensor.matmul(
                                    pt[:],
                                    mts[(k, s)][:, wt * 128 : (wt + 1) * 128],
                                    rhs,
                                    start=(i == 0),
                                    stop=(i == len(ks) - 1),
                                )
                            pts.append(pt)
                        col = s * WT + wt
                        # PSUM eviction + 1/m scale split across DVE and ACT
                        # (a single engine's 32-op eviction chain paces the
                        # whole tail at ~20us; HW-traced)
                        nc.vector.tensor_scalar_mul(
                            ot[:, wt * E : wt * E + 384],
                            pts[0][:], msct[:, col : col + 1]
                        )
                        nc.scalar.mul(
                            ot[:, wt * E + 384 : (wt + 1) * E],
                            pts[1][:], msct[:, col : col + 1]
                        )
                    nc.sync.dma_start(
                        out=y[s * 256 : (s + 1) * 256, :].rearrange(
                            "(c p) e -> p c e", p=128
                        ),
                        in_=ot[:].rearrange("p (c e) -> p c e", e=E),
                    )
    nc.compile()
    return nc


def _build_nc_acc(reps, bufs, dt="f32", nchunk=4):
    """Native indirect gathers, zero compute engines: host uploads x/2, the
    B gather CCE-accumulates onto the A gather in SBUF, and the only
    consumer is a DMA store (the HW-verified-safe consumer)."""
    _DT = _F32 if dt == "f32" else _BF16
    cpw = NWP // nchunk
    wpp = cpw // 128
    nc = bacc.Bacc(
        "TRN2", target_bir_lowering=False, debug=False, num_devices=NCORES
    )
    x = nc.dram_tensor("x", [ROWS, E], _DT, kind="ExternalInput").ap()
    idx = nc.dram_tensor("idx", [128, 2 * wpp * nchunk], mybir.dt.int32,
                         kind="ExternalInput").ap()
    y = nc.dram_tensor("y", [NWP, E], _DT, kind="ExternalOutput").ap()
    with tile.TileContext(nc) as tc:
        with (
            tc.tile_pool(name="idxp", bufs=1) as ipool,
            tc.tile_pool(name="io", bufs=bufs) as pool,
        ):
            itall = ipool.tile([128, 2 * wpp * nchunk], mybir.dt.int32,
                               tag="itall")
            nc.sync.dma_start(out=itall[:], in_=idx[:, :])
            for _rep in range(reps):
                for q in range(nchunk):
                    o = 2 * wpp * q
                    t = pool.tile([128, wpp * E], _DT, tag="t")
                    t3 = t[:].rearrange("p (c e) -> p c e", e=E)
                    nc.gpsimd.indirect_dma_start(
                        out=t3, out_offset=None, in_=x[:, :],
                        in_offset=bass.IndirectOffsetOnAxis(
                            ap=itall[:, o : o + wpp], axis=0),
                    )
                    nc.gpsimd.indirect_dma_start(
                        out=t3, out_offset=None, in_=x[:, :],
                        in_offset=bass.IndirectOffsetOnAxis(
                            ap=itall[:, o + wpp : o + 2 * wpp], axis=0),
                        compute_op=mybir.AluOpType.add,
                    )
                    nc.sync.dma_start(
                        out=y[q * cpw : (q + 1) * cpw, :].rearrange(
                            "(p j) e -> p (j e)", p=128),
                        in_=t[:],
                    )
    nc.compile()
    return nc


def _build_nc_sized(reps, bufs, sizes, hscale):
    """ab-mode kernel with per-chunk word counts `sizes` (multiples of 128
    summing to NWP). Small leading chunks start transfers sooner; small
    trailing chunks shorten the exposed compute+store tail."""
    assert sum(sizes) == NWP and all(s % 128 == 0 for s in sizes)
    nc = bacc.Bacc(
        "TRN2", target_bir_lowering=False, debug=False, num_devices=NCORES
    )
    x = nc.dram_tensor("x", [ROWS, E], _F32, kind="ExternalInput").ap()
    total_cols = sum(2 * s // 16 for s in sizes)
    idx = nc.dram_tensor("idx", [128, total_cols], _I16, kind="ExternalInput").ap()
    y = nc.dram_tensor("y", [NWP, E], _F32, kind="ExternalOutput").ap()
    jmax = max(sizes) // 128

    with tile.TileContext(nc) as tc:
        with (
            tc.tile_pool(name="idxp", bufs=1) as ipool,
            tc.tile_pool(name="io", bufs=bufs) as pool,
        ):
            itall = ipool.tile([128, total_cols], _I16, tag="itall")
            nc.sync.dma_start(out=itall[:], in_=idx[:, :])
            for _rep in range(reps):
                off_c = 0
                off_w = 0
                for s in sizes:
                    jj = s // 128
                    cols = 2 * s // 16
                    t = pool.tile([128, 2 * jmax * E], _F32, tag="t")
                    nc.gpsimd.dma_gather(
                        t[:, : 2 * jj * E].rearrange("p (c e) -> p c e", e=E),
                        x[:, :],
                        itall[:, off_c : off_c + cols],
                        2 * s,
                        2 * s,
                        E,
                    )
                    c = pool.tile([128, jmax * E], _F32, tag="c")
                    nc.vector.tensor_add(
                        out=c[:, : jj * E],
                        in0=t[:, : jj * E],
                        in1=t[:, jj * E : 2 * jj * E],
                    )
                    if not hscale:
                        nc.scalar.mul(c[:, : jj * E], c[:, : jj * E], 0.5)
                    nc.sync.dma_start(
                        out=y[off_w : off_w + s, :].rearrange(
                            "(p j) e -> p (j e)", p=128
                        ),
                        in_=c[:, : jj * E],
                    )
                    off_c += cols
                    off_w += s
    nc.compile()
    return nc


def _build_nc_jm(tmaxs, reps=1, split_first=2, pbufs=2, obufs=4, mbufs=3):
    """JIT-specialized segment-sum matmul (mode "jm").

    Value-specialization: tmaxs[s] = max total BPE rows (sum of mappings)
    over the 8 cores' sentences assigned to slot s (host sorts sentences by
    row count and deals groups of 8 across cores, so the per-slot envelope
    is tight).  Loads read only rows [1, 1+tmaxs[s]) of each sentence
    (~383 of 512 avg) -- used rows are a contiguous prefix after [CLS].
    Word tile wt0 (words 0..127) spans rows [0, 256) hard (sum of 128
    mappings <= 256), wt1 spans [128, tmax): 4-5 (ktile x wtile) matmul
    pairs per sentence vs 6 in mode "mm".

    The 0/1 segment matrix is built per (ktile, wtile) block from an
    uploaded fp16 boundary row: one batched is_le compare + one batched
    adjacent-diff per sentence (2 DVE ops).  Boundary replication to 128
    partitions rides the PE (ones-matmul) interleaved 2 sentences ahead of
    the main matmul stream, so the PE never idles on it.  Rows beyond a
    core's own sentence length up to the slot envelope get weight 0 from
    the compare (boundaries are per-core runtime data; only AP shapes are
    baked), so the SPMD program stays core-uniform.
    """
    NS = BPC
    SW = 257                      # boundaries s_0..s_255 + sentinel per sentence
    nc = bacc.Bacc(
        "TRN2", target_bir_lowering=False, debug=False, num_devices=NCORES
    )
    x = nc.dram_tensor("x", [ROWS, E], _BF16, kind="ExternalInput").ap()
    sth = nc.dram_tensor("sth", [1, NS * SW], mybir.dt.float16,
                         kind="ExternalInput").ap()
    # aux[:, 0:4] = iota(p) + 128*kt ; aux[:, 4:20] = 1/m per (sent, wtile)
    aux = nc.dram_tensor("aux", [128, 20], _F32, kind="ExternalInput").ap()
    y = nc.dram_tensor("y", [NS * 256, E], _BF16, kind="ExternalOutput").ap()

    geo = []
    for s in range(NS):
        T = tmaxs[s]
        kt_n = (T + 127) // 128
        nfull = T // 128
        rem = T % 128
        pairs = ([(kt, 0) for kt in range(min(2, kt_n))]
                 + [(kt, 1) for kt in range(1, kt_n)])
        geo.append((T, kt_n, nfull, rem, pairs))
    nbmax = max(len(g[4]) for g in geo)

    with tile.TileContext(nc) as tc:
        with (
            tc.tile_pool(name="const", bufs=1) as cpool,
            tc.tile_pool(name="xs", bufs=1) as xpool,
            tc.tile_pool(name="m", bufs=mbufs) as mpool,
            tc.tile_pool(name="ps", bufs=pbufs, space="PSUM") as ppool,
            tc.tile_pool(name="out", bufs=obufs) as opool,
        ):
            # x loads first: no deps, 2 injection rings (ACT HWDGE + SWDGE).
            # First split_first sentences load per-ktile so the first
            # matmul unblocks on one 196KB transfer, not a whole sentence.
            xts = []
            for s in range(NS):
                T, kt_n, nfull, rem, pairs = geo[s]
                eng = nc.scalar if s % 2 == 0 else nc.gpsimd
                xt = xpool.tile([128, kt_n * E], _BF16, tag=f"x{s}")
                base = s * S + 1
                if s < split_first:
                    for kt in range(nfull):
                        eng.dma_start(
                            out=xt[:, kt * E : (kt + 1) * E],
                            in_=x[base + 128 * kt : base + 128 * (kt + 1), :],
                        )
                elif nfull:
                    eng.dma_start(
                        out=xt[:, : nfull * E].rearrange(
                            "p (k e) -> p k e", e=E),
                        in_=x[base : base + 128 * nfull, :].rearrange(
                            "(k p) e -> p k e", p=128),
                    )
                if rem:
                    eng.dma_start(
                        out=xt[:rem, nfull * E :],
                        in_=x[base + 128 * nfull : base + T, :],
                    )
                xts.append(xt)
            sfl = cpool.tile([1, NS * SW], mybir.dt.float16, tag="sfl")
            nc.sync.dma_start(out=sfl[:], in_=sth[:, :])
            auxt = cpool.tile([128, 20], _F32, tag="aux")
            nc.sync.dma_start(out=auxt[:], in_=aux[:, :])
            gt = auxt[:, 0:4]
            mst = auxt[:, 4:20]
            ones = cpool.tile([1, 128], mybir.dt.float16, tag="ones")
            nc.vector.memset(ones[:], 1.0)
            stht = cpool.tile([128, NS * SW], mybir.dt.float16, tag="stht")

            def _repl(s):
                # replicate slot s boundaries to 128 partitions via PE
                pb = ppool.tile([128, 384], _F32, tag="p00")
                nc.tensor.matmul(pb[:, :SW], ones[:],
                                 sfl[:, s * SW : (s + 1) * SW],
                                 start=True, stop=True)
                nc.vector.tensor_copy(out=stht[:, s * SW : (s + 1) * SW],
                                      in_=pb[:, :SW])

            for _rep in range(reps):
                _repl(0)
                _repl(1)
                for s in range(NS):
                    if s + 2 < NS:
                        _repl(s + 2)
                    T, kt_n, nfull, rem, pairs = geo[s]
                    nb = len(pairs)
                    ct = mpool.tile([128, nbmax * 129], _BF16, tag="ct")
                    for bi, (kt, wt) in enumerate(pairs):
                        nc.vector.tensor_scalar(
                            ct[:, bi * 129 : (bi + 1) * 129],
                            stht[:, s * SW + 128 * wt : s * SW + 128 * wt + 129],
                            gt[:, kt : kt + 1], None,
                            op0=mybir.AluOpType.is_le,
                        )
                    mts = mpool.tile([128, nbmax * 128], _BF16, tag="mt")
                    ctv = ct[:, : nb * 129].rearrange("p (b c) -> p b c", c=129)
                    mtv = mts[:, : nb * 128].rearrange("p (b c) -> p b c", c=128)
                    nc.vector.tensor_tensor(out=mtv, in0=ctv[:, :, 0:128],
                                            in1=ctv[:, :, 1:129],
                                            op=mybir.AluOpType.subtract)
                    ot = opool.tile([128, 2 * E], _BF16, tag="o")
                    for wt in range(2):
                        wps = [(bi, kt) for bi, (kt, w) in enumerate(pairs)
                               if w == wt]
                        pts = [ppool.tile([128, 384], _F32, tag=f"p{wt}{h}",
                                          name=f"pt{wt}{h}")
                               for h in range(2)]
                        for i, (bi, kt) in enumerate(wps):
                            wk = rem if (rem and kt == kt_n - 1) else 128
                            for h in range(2):
                                nc.tensor.matmul(
                                    pts[h][:],
                                    mts[:wk, bi * 128 : (bi + 1) * 128],
                                    xts[s][:wk, kt * E + h * 384
                                           : kt * E + (h + 1) * 384],
                                    start=(i == 0), stop=(i == len(wps) - 1),
                                )
                        col = 4 + 2 * s + wt
                        nc.vector.tensor_scalar_mul(
                            ot[:, wt * E : wt * E + 384], pts[0][:],
                            auxt[:, col : col + 1])
                        nc.scalar.mul(
                            ot[:, wt * E + 384 : (wt + 1) * E], pts[1][:],
                            auxt[:, col : col + 1])
                    nc.sync.dma_start(
                        out=y[s * 256 : (s + 1) * 256, :].rearrange(
                            "(c p) e -> p c e", p=128),
                        in_=ot[:].rearrange("p (c e) -> p c e", e=E),
                    )
    nc.compile()
    return nc


def _make_in_maps_jm(output, mappings):
    """Host prep for mode "jm": sort sentences by total BPE rows, deal
    groups of 8 across cores (slot s, core k <- sentence order[s*8+k]) so
    each slot's row envelope is tight; upload bf16 x in slot order plus
    fp16 boundary rows and f32 aux (iota+128kt cols, 1/m scales)."""
    import ml_dtypes

    NS = BPC
    SW = 257
    output = np.asarray(output)
    mappings = np.asarray(mappings, dtype=np.int32)
    ends = np.cumsum(mappings, axis=1, dtype=np.int32)      # [B, W]
    Ti = ends[:, -1]                                        # [B]
    order = np.argsort(Ti, kind="stable")
    assign = order.reshape(NS, NCORES)                      # [slot, core]
    tmaxs = tuple(int(Ti[assign[s]].max()) for s in range(NS))
    xbf = output.astype(ml_dtypes.bfloat16)
    aux_base = np.empty((128, 20), np.float32)
    aux_base[:, 0:4] = (np.arange(128, dtype=np.float32)[:, None]
                        + 128.0 * np.arange(4, dtype=np.float32)[None, :])
    minv = 1.0 / mappings.astype(np.float32)
    in_maps = []
    for k in range(NCORES):
        sents = assign[:, k]
        xk = np.ascontiguousarray(xbf[sents].reshape(ROWS, E))
        sthk = np.zeros((NS, SW), np.float16)
        sthk[:, 1:256] = ends[sents]
        sthk[:, 256] = Ti[sents]
        auxk = aux_base.copy()
        auxk[:, 4:20] = 1.0
        mi = minv[sents]
        for s in range(NS):
            auxk[:, 4 + 2 * s] = mi[s, 0:128]
            auxk[:127, 4 + 2 * s + 1] = mi[s, 128:255]
        in_maps.append({
            "x": xk,
            "sth": np.ascontiguousarray(sthk.reshape(1, -1)),
            "aux": np.ascontiguousarray(auxk),
        })
    return in_maps, assign, tmaxs


def _run_jm(output, mappings, reps=1, split_first=2, pbufs=2, obufs=4,
            mbufs=3, **kw):
    in_maps, assign, tmaxs = _make_in_maps_jm(output, mappings)
    key = ("jm", tmaxs, reps, split_first, pbufs, obufs, mbufs)
    if key not in _NC:
        _NC[key] = _build_nc_jm(tmaxs, reps, split_first, pbufs, obufs, mbufs)
    res = run_bass_kernel_spmd(_NC[key], in_maps, list(range(NCORES)), **kw)
    full = np.empty((B, W, E), np.float32)
    for k, r in enumerate(res.results):
        yk = np.asarray(r["y"], dtype=np.float32).reshape(BPC, 256, E)[:, :W]
        full[assign[:, k]] = yk
    return full, res


_NC = {}


def _get_nc(reps=1, bufs=2, order="pc", nq=1, mode="ab", nchunk=NCHUNK,
            merged_idx=False, warm=False, hscale=False, sizes=None,
            dt="f32", split0=False):
    key = (reps, bufs, order, nq, mode, nchunk, merged_idx, warm, hscale,
           tuple(sizes) if sizes else None, dt, split0)
    if key not in _NC:
        _NC[key] = _build_nc(reps, bufs, order, nq, mode, nchunk, merged_idx,
                             warm, hscale, sizes, dt, split0)
    return _NC[key]


def _wrap16(flat):
    """int16 index list -> [128, n/16] wrapped (i -> [i%16, i//16]) + 8x rep."""
    return np.tile(flat.reshape(-1, 16).T, (8, 1)).astype(np.int16)


def _make_in_maps(output, mappings, order="pc", mode="ab", nchunk=NCHUNK,
                  hscale=False, sizes=None, dt="f32"):
    output = np.asarray(output)
    if hscale:
        # fold the *0.5 of the span mean into the shard upload: a/2 + b/2
        # rounds identically to (a+b)/2 in f32 (halving is exact).
        output = output * np.float32(0.5)
    npdt = np.float32 if dt == "f32" else _np_bf16()
    mappings = np.asarray(mappings, dtype=np.int32)
    ends = np.cumsum(mappings, axis=1, dtype=np.int32)  # [B, W] exclusive ends
    src_a = ends - mappings + 1                         # +1: skip [CLS]
    src_b = ends                                        # (e-1) + 1
    if mode in ("wh", "wind"):
        # scale every BPE row by 1/m of its owning word (0.5/1 exact in f32,
        # single bf16 rounding after); junk rows keep scale 1 and are killed
        # on-device by beta=0.
        g = np.ones((B, S), np.float32)
        two = mappings == 2
        np.put_along_axis(
            g, src_a, np.where(two, np.float32(0.5), np.float32(1.0)), axis=1
        )
        bi, wi_ = np.nonzero(two)
        g[bi, src_a[bi, wi_] + 1] = 0.5
        output = output * g[:, :, None]
    output = np.ascontiguousarray(output.astype(npdt))

    if mode == "mm":
        in_maps = []
        gcol = (np.arange(128, dtype=np.float32)[:, None]
                + 128.0 * np.arange(4, dtype=np.float32)[None, :])
        gcol = np.ascontiguousarray(gcol)
        minv = 1.0 / mappings.astype(np.float32)          # [B, W]
        for k in range(NCORES):
            bs = slice(k * BPC, (k + 1) * BPC)
            sa = src_a[bs]                                # [8, 255] s-coords
            sth = np.empty((BPC, 257), np.float16)
            sth[:, :255] = sa
            sth[:, 255] = ends[bs, -1] + 1                # pad word start
            sth[:, 256] = 600.0                           # sentinel > 511
            sth = np.ascontiguousarray(sth.reshape(1, -1))  # [1, 2056]
            msc = np.ones((128, BPC * 2), np.float32)
            mi = minv[bs]                                 # [8, 255]
            for s_ in range(BPC):
                msc[:, s_ * 2] = mi[s_, 0:128]
                msc[:128 - 1, s_ * 2 + 1] = mi[s_, 128:255]
            xk = np.ascontiguousarray(
                output[bs].reshape(ROWS, E).astype(_np_bf16()))
            in_maps.append({"x": xk, "sth": sth, "gcol": gcol,
                            "msc": np.ascontiguousarray(msc)})
        return in_maps

    in_maps = []
    for k in range(NCORES):
        bs = slice(k * BPC, (k + 1) * BPC)
        base = (np.arange(BPC, dtype=np.int32) * S)[:, None]
        a = (src_a[bs] + base).reshape(-1)
        b = (src_b[bs] + base).reshape(-1)
        pad = np.zeros(NWP - NW, np.int32)
        a = np.concatenate([a, pad])  # [NWP] word-ordered flat row ids
        b = np.concatenate([b, pad])
        x = np.ascontiguousarray(output[bs].reshape(ROWS, E))
        if mode == "wind":
            wpp = NWP // nchunk // 128
            mm = np.concatenate(
                [mappings[bs].reshape(-1), np.ones(NWP - NW, np.int32)]
            )
            beta = (mm == 2).astype(npdt)
            # [p, q*wpp+j] = element offset of word q*cpw + p*wpp + j
            aw = a.reshape(nchunk, 128, wpp).transpose(1, 0, 2)  # [p, q, j]
            idx = np.ascontiguousarray(aw.reshape(128, -1).astype(np.int32))
            bt = np.empty((nchunk, 128, wpp), npdt)
            cpw_ = NWP // nchunk
            for q in range(nchunk):
                sl = slice(q * cpw_, (q + 1) * cpw_)
                bt[q] = beta[sl].reshape(128, wpp)
            in_maps.append({"x": x, "idx": idx, "bt": bt})
            continue
        if mode in ("ind", "acc"):
            nck = nchunk
            wpp = NWP // nck // 128
            ia = a.reshape(nck, 128, wpp).transpose(1, 0, 2)  # [p, q, j]
            ib = b.reshape(nck, 128, wpp).transpose(1, 0, 2)
            idx = np.concatenate(
                [np.concatenate([ia[:, q], ib[:, q]], axis=1)
                 for q in range(nck)],
                axis=1,
            ).astype(np.int32)  # [128, 2*wpp*nck], cols 2*wpp*q + j
            in_maps.append({"x": x, "idx": np.ascontiguousarray(idx)})
            continue
        if sizes is not None:
            segs = []
            off = 0
            for s in sizes:
                jj = s // 128
                aq = a[off : off + s].reshape(128, jj).T.ravel()
                bq = b[off : off + s].reshape(128, jj).T.ravel()
                segs.append(_wrap16(np.concatenate([aq, bq])))
                off += s
            in_maps.append({"x": x, "idx": np.concatenate(segs, axis=1)})
            continue
        cpw = NWP // nchunk
        jj = cpw // 128
        if mode == "ab":
            idx = np.empty((nchunk, 128, 2 * cpw // 16), np.int16)
            for q in range(nchunk):
                aq = a[q * cpw : (q + 1) * cpw]
                bq = b[q * cpw : (q + 1) * cpw]
                if order == "pc":
                    # gathered i = c*128 + p holds word q*cpw + p*jj + c
                    aq = aq.reshape(128, jj).T.ravel()
                    bq = bq.reshape(128, jj).T.ravel()
                # 'seq': gathered i holds word q*cpw + i (ascending rows)
                idx[q] = _wrap16(np.concatenate([aq, bq]))
            in_maps.append({"x": x, "idx": idx})
        elif mode == "wh":
            mm = np.concatenate(
                [mappings[bs].reshape(-1), np.ones(NWP - NW, np.int32)]
            )
            beta = (mm == 2).astype(npdt)
            idx = np.empty((nchunk, 128, cpw // 16), np.int16)
            bt = np.empty((nchunk, 128, jj), npdt)
            for q in range(nchunk):
                sl = slice(q * cpw, (q + 1) * cpw)
                idx[q] = _wrap16(a[sl].reshape(128, jj).T.ravel())
                bt[q] = beta[sl].reshape(128, jj)
            in_maps.append({"x": x, "idx": idx, "bt": bt})
        else:
            m = np.concatenate(
                [mappings[bs].reshape(-1), np.ones(NWP - NW, np.int32)]
            ).astype(np.float32)
            r1 = 1.0 / m
            r2 = (m - 1.0) / m
            rdt = np.float32 if mode == "ws" else npdt
            idx = np.empty((nchunk, 128, cpw // 16), np.int16)
            rw = np.empty((nchunk, 128, 2 * jj), rdt)
            for q in range(nchunk):
                sl = slice(q * cpw, (q + 1) * cpw)
                aq = a[sl].reshape(128, jj).T.ravel()  # i = c*128 + p
                idx[q] = _wrap16(aq)
                rw[q, :, 0::2] = r1[sl].reshape(128, jj).astype(rdt)
                rw[q, :, 1::2] = r2[sl].reshape(128, jj).astype(rdt)
            in_maps.append({"x": x, "idx": idx, "rw": rw})
    return in_maps


def _run(output, mappings, reps=1, bufs=2, order="pc", nq=1, mode="ab",
         nchunk=NCHUNK, merged_idx=False, warm=False, hscale=False,
         sizes=None, dt="f32", split0=False, split_first=2, pbufs=2,
         obufs=4, mbufs=3, **kw):
    if mode == "jm":
        return _run_jm(output, mappings, reps=reps, split_first=split_first,
                       pbufs=pbufs, obufs=obufs, mbufs=mbufs, **kw)
    in_maps = _make_in_maps(output, mappings, order, mode, nchunk, hscale,
                            sizes, dt)
    nc = _get_nc(reps, bufs, order, nq, mode, nchunk, merged_idx, warm,
                 hscale, sizes, dt, split0)
    res = run_bass_kernel_spmd(nc, in_maps, list(range(NCORES)), **kw)
    if mode == "mm":
        outs = [
            np.asarray(r["y"], dtype=np.float32).reshape(BPC, 256, E)[:, :W]
            for r in res.results
        ]
    else:
        outs = [
            np.asarray(r["y"][:NW], dtype=np.float32).reshape(BPC, W, E)
            for r in res.results
        ]
    return np.concatenate(outs, axis=0), res


# Best HW-verified configuration: JIT-specialized matmul segment-sum
# (mode "jm").  Prior best: mode "mm" @ ~42.4us median; runner-up kept
# working: dict(bufs=6, order="pc", nq=1, mode="wh", nchunk=8,
# merged_idx=True, split0=True, dt="bf16") @ ~52.7us.
_CFG = dict(mode="jm")


def kernel(output, mappings):
    full, _ = _run(output, mappings, **_CFG)
    return full



# revision 9
# speedup vs baseline: 1.1455x; 1.1455x over previous
"""BERT per-word mean-pool (segment reduce) on 8 Trainium2 NeuronCores.

Problem: output[B=64, S=512, E=768] f32, mappings[B, W=255] int32 (values 1 or 2).
Per sentence, strip [CLS]/[SEP], mean-pool contiguous BPE spans into word vectors.

Key identity: every word's span has 1 or 2 BPE rows.  With s = span start,
    out[w] = (1/m) * (hs rows s .. s+m-1 summed),  m in {1, 2}.

Sharding: pure data parallel, 8 sentences per core, no cross-core comms.
All device data in bf16 (tolerance 2e-2 >> bf16's ~2e-3); host casts.

Two competitive kernels, HW-measured:

* mode "mm" (default): segment-sum as matmul. NO gathers, NO Q7 ucode --
  dodges the ~16us ucode-ready gate that floors every dma_gather kernel.
  Contiguous loads of x (SWDGE+HWDGE alternating); the 0/1 segment matrix
  M^T[t, w] = [s_w <= t] - [s_{w+1} <= t] is built on-chip from an uploaded
  boundary row (2 DVE ops per 128-row k-tile); TensorE accumulates
  out = M^T.T @ x per (sentence, 128-word tile) in PSUM; ACT evicts with a
  per-partition 1/m scale; HWDGE stores.

* mode "wh": windowed ucode gather. One 2-row-window descriptor per word
  (InstDMAGatherAnt), x pre-scaled by 1/m on host, single fused DVE
  scalar_tensor_tensor per word-column: out = t1*[m==2] + t0.

Raw indirect InstDMACopy (modes ind/acc/wind) is broken on this runtime:
the DGE consumes ONE offset per partition of the out AP and fetches
consecutive rows after it (HW-probed); a 3D out AP makes it worse. Do not
use those modes.
"""

import numpy as np

from concourse import bacc, bass, mybir, tile
from concourse.bass_utils import run_bass_kernel_spmd

B, S, W, E = 64, 512, 255, 768
NCORES = 8
BPC = B // NCORES            # sentences per core
NW = BPC * W                 # 2040 real words per core
NWP = 2048                   # padded word count (multiple of 512)
NCHUNK = 4                   # chunks per core
CPW = NWP // NCHUNK          # 512 words per chunk
JJ = CPW // 128              # 4 words per partition per chunk
ROWS = BPC * S               # 4096 input rows per core
NIDX = 2 * CPW               # 1024 gather indices per chunk (A then B)

_F32 = mybir.dt.float32
_BF16 = mybir.dt.bfloat16
_I16 = mybir.dt.int16

_NPBF16 = None


def _np_bf16():
    global _NPBF16
    if _NPBF16 is None:
        import ml_dtypes

        _NPBF16 = ml_dtypes.bfloat16
    return _NPBF16


def _build_nc(reps=1, bufs=2, order="pc", nq=1, mode="ab", nchunk=NCHUNK,
              merged_idx=False, warm=False, hscale=False, sizes=None,
              dt="f32", split0=False):
    _DT = _F32 if dt == "f32" else _BF16
    if sizes is not None:
        return _build_nc_sized(reps, bufs, sizes, hscale)
    if mode == "ind":
        return _build_nc_ind(reps, bufs, dt, nchunk, hscale)
    if mode == "acc":
        return _build_nc_acc(reps, bufs, dt, nchunk)
    if mode == "wind":
        return _build_nc_wind(reps, bufs, dt, nchunk)
    if mode == "mm":
        return _build_nc_mm(reps)
    nc = bacc.Bacc(
        "TRN2",
        target_bir_lowering=False,
        debug=False,
        num_devices=NCORES,
        num_swdge_queues=nq,
    )
    x = nc.dram_tensor("x", [ROWS, E], _DT, kind="ExternalInput").ap()
    # indices are int16, wrapped [i%16, i//16] into 16 partitions and
    # replicated 8x down to 128 partitions (Q7 core replication).
    cpw = NWP // nchunk
    jj = cpw // 128
    nidx = 2 * cpw if mode == "ab" else cpw
    idx = nc.dram_tensor(
        "idx", [nchunk, 128, nidx // 16], _I16, kind="ExternalInput"
    ).ap()
    _RDT = _F32 if mode == "ws" else _DT  # tensor_scalar wants f32 scalars
    if mode in ("win", "ws"):
        # per word w: rw[p, 2c] = 1/m(w), rw[p, 2c+1] = (m(w)-1)/m(w)
        rw = nc.dram_tensor(
            "rw", [nchunk, 128, 2 * jj], _RDT, kind="ExternalInput"
        ).ap()
    if mode == "wh":
        # per word w: bt[p, c] = [m(w) == 2]; x rows pre-scaled by 1/m on host
        bt = nc.dram_tensor(
            "bt", [nchunk, 128, jj], _DT, kind="ExternalInput"
        ).ap()
    y = nc.dram_tensor("y", [NWP, E], _DT, kind="ExternalOutput").ap()

    with tile.TileContext(nc) as tc:
        with (
            tc.tile_pool(name="idxp", bufs=1) as ipool,
            tc.tile_pool(name="io", bufs=bufs) as pool,
        ):
            if warm:
                # dummy 16-index gather issued first: triggers the Q7
                # ucode IRAM fetch (~6us) while the idx loads stream in,
                # so the first real gather isn't stalled on it.
                wi = ipool.tile([128, 1], _I16, tag="warmi")
                nc.gpsimd.memset(wi[:], 0)
                wo = ipool.tile([128, E], _DT, tag="warmo")
                nc.gpsimd.dma_gather(
                    wo[:].rearrange("p (c e) -> p c e", e=E),
                    x[:, :], wi[:], 16, 16, E,
                )
            its, rts = [], []
            ncols = nidx // 16
            if merged_idx:
                its = []
                if split0:
                    # chunk-0 idx as its own tiny first DMA so the first
                    # gather isn't gated on the full idx upload
                    it0 = ipool.tile([128, ncols], _I16, tag="it0")
                    nc.sync.dma_start(out=it0[:], in_=idx[0])
                    its.append(it0[:])
                    itall = ipool.tile(
                        [128, (nchunk - 1) * ncols], _I16, tag="itall"
                    )
                    nc.sync.dma_start(
                        out=itall[:].rearrange(
                            "p (q s) -> p q s", q=nchunk - 1
                        ),
                        in_=idx[1:].rearrange("q p s -> p q s"),
                    )
                    its += [
                        itall[:, q * ncols : (q + 1) * ncols]
                        for q in range(nchunk - 1)
                    ]
                else:
                    itall = ipool.tile([128, nchunk * ncols], _I16, tag="itall")
                    nc.sync.dma_start(
                        out=itall[:].rearrange("p (q s) -> p q s", q=nchunk),
                        in_=idx.rearrange("q p s -> p q s"),
                    )
                    its = [
                        itall[:, q * ncols : (q + 1) * ncols]
                        for q in range(nchunk)
                    ]
            else:
                for q in range(nchunk):
                    it = ipool.tile([128, ncols], _I16, tag=f"it{q}")
                    nc.sync.dma_start(out=it[:], in_=idx[q])
                    its.append(it[:])
            if mode in ("win", "ws"):
                # single merged weight load (one HWDGE DMA for all chunks)
                rtall = ipool.tile([128, nchunk * 2 * jj], _RDT, tag="rtall")
                nc.sync.dma_start(
                    out=rtall[:].rearrange("p (q s) -> p q s", q=nchunk),
                    in_=rw.rearrange("q p s -> p q s"),
                )
                rts = [
                    rtall[:, q * 2 * jj : (q + 1) * 2 * jj]
                    for q in range(nchunk)
                ]
            if mode == "wh":
                btall = ipool.tile([128, nchunk * jj], _DT, tag="btall")
                nc.sync.dma_start(
                    out=btall[:].rearrange("p (q s) -> p q s", q=nchunk),
                    in_=bt.rearrange("q p s -> p q s"),
                )
            for _rep in range(reps):
                for q in range(nchunk):
                    if mode == "ab":
                        # gathered slot i -> T[i % 128, i // 128, :]
                        # i = c*128 + p:  c in 0..3 -> first-BPE row of word
                        # w = q*512 + p*4 + c;  c in 4..7 -> last-BPE row.
                        t = pool.tile([128, 2 * jj * E], _DT, tag="t")
                        nc.gpsimd.dma_gather(
                            t[:].rearrange("p (c e) -> p c e", e=E),
                            x[:, :],
                            its[q],
                            nidx,
                            nidx,
                            E,
                            queue_num=q % nq,
                        )
                        c = pool.tile([128, jj * E], _DT, tag="c")
                        nc.vector.tensor_add(
                            out=c[:], in0=t[:, : jj * E], in1=t[:, jj * E :]
                        )
                        if not hscale:
                            nc.scalar.mul(c[:], c[:], 0.5)
                    else:
                        # one 2-row window [s, s+2) per word, 6KB descriptors;
                        # out[w] = win[0]*r1 + win[1]*r2 kills the junk row
                        # (m=1: r=(1,0); m=2: r=(.5,.5)).
                        t = pool.tile([128, 2 * jj * E], _DT, tag="t")
                        xw = bass.AP(x.tensor, 0, [[E, ROWS - 1], [1, 2 * E]])
                        nc.gpsimd.dma_gather(
                            t[:].rearrange("p (c e) -> p c e", e=2 * E),
                            xw,
                            its[q],
                            cpw,
                            cpw,
                            2 * E,
                            elem_step=E,
                            queue_num=q % nq,
                        )
                        c = pool.tile([128, jj * E], _DT, tag="c")
                        if mode == "wh":
                            # x rows pre-scaled by 1/m on host; one fused DVE
                            # op per word-column: out = t1*[m==2] + t0, all
                            # APs contiguous [128, E] (full-rate DVE).
                            for j in range(jj):
                                nc.vector.scalar_tensor_tensor(
                                    out=c[:, j * E : (j + 1) * E],
                                    in0=t[:, (2 * j + 1) * E : (2 * j + 2) * E],
                                    scalar=btall[:, q * jj + j : q * jj + j + 1],
                                    in1=t[:, 2 * j * E : (2 * j + 1) * E],
                                    op0=mybir.AluOpType.mult,
                                    op1=mybir.AluOpType.add,
                                )
                        elif mode == "ws":
                            # device weights, contiguous slices: per word-col
                            # c3 = t1*r2; c = t0*r1 + c3
                            c3 = pool.tile([128, jj * E], _DT, tag="c3")
                            for j in range(jj):
                                nc.vector.tensor_scalar_mul(
                                    c3[:, j * E : (j + 1) * E],
                                    t[:, (2 * j + 1) * E : (2 * j + 2) * E],
                                    rts[q][:, 2 * j + 1 : 2 * j + 2],
                                )
                                nc.vector.scalar_tensor_tensor(
                                    out=c[:, j * E : (j + 1) * E],
                                    in0=t[:, 2 * j * E : (2 * j + 1) * E],
                                    scalar=rts[q][:, 2 * j : 2 * j + 1],
                                    in1=c3[:, j * E : (j + 1) * E],
                                    op0=mybir.AluOpType.mult,
                                    op1=mybir.AluOpType.add,
                                )
                        else:
                            t3 = t[:].rearrange("p (c e) -> p c e", e=2 * E)
                            r3 = rts[q].rearrange("p (c f) -> p c f", f=2)
                            c3 = pool.tile([128, jj * E], _DT, tag="c3")
                            cv = c[:].rearrange("p (j e) -> p j e", e=E)
                            c3v = c3[:].rearrange("p (j e) -> p j e", e=E)
                            nc.vector.tensor_tensor(
                                out=cv,
                                in0=t3[:, :, :E],
                                in1=r3[:, :, 0:1].to_broadcast([128, jj, E]),
                                op=mybir.AluOpType.mult,
                            )
                            nc.vector.tensor_tensor(
                                out=c3v,
                                in0=t3[:, :, E:],
                                in1=r3[:, :, 1:2].to_broadcast([128, jj, E]),
                                op=mybir.AluOpType.mult,
                            )
                            nc.vector.tensor_add(out=c[:], in0=c[:], in1=c3[:])
                    ychunk = y[q * cpw : (q + 1) * cpw, :]
                    if order == "pc":
                        nc.sync.dma_start(
                            out=ychunk.rearrange("(p j) e -> p (j e)", p=128),
                            in_=c[:],
                        )
                    else:
                        nc.sync.dma_start(
                            out=ychunk.rearrange("(j p) e -> p j e", p=128),
                            in_=c[:].rearrange("p (j e) -> p j e", e=E),
                        )
    nc.compile()
    return nc


def _build_nc_ind(reps, bufs, dt="f32", nchunk=4, hscale=False, tbufs=1):
    """Native SWDGE indirect gather (no ucode library -> no ~16us Q7 ucode
    ready gate). One InstDMACopy per chunk reading a column slice of a single
    int32 offset tile; slices advance monotonically (the HW-verified-safe
    pattern). Per chunk: first wpp cols = first-BPE rows, next wpp cols =
    last-BPE rows of words w = q*cpw + p*wpp + j."""
    _DT = _F32 if dt == "f32" else _BF16
    cpw = NWP // nchunk
    wpp = cpw // 128              # words per partition per chunk
    nc = bacc.Bacc(
        "TRN2", target_bir_lowering=False, debug=False, num_devices=NCORES
    )
    x = nc.dram_tensor("x", [ROWS, E], _DT, kind="ExternalInput").ap()
    idx = nc.dram_tensor("idx", [128, 2 * wpp * nchunk], mybir.dt.int32,
                         kind="ExternalInput").ap()
    y = nc.dram_tensor("y", [NWP, E], _DT, kind="ExternalOutput").ap()
    with tile.TileContext(nc) as tc:
        with (
            tc.tile_pool(name="idxp", bufs=1) as ipool,
            tc.tile_pool(name="tp", bufs=tbufs) as tpool,
            tc.tile_pool(name="io", bufs=bufs) as pool,
        ):
            itall = ipool.tile([128, 2 * wpp * nchunk], mybir.dt.int32,
                               tag="itall")
            nc.sync.dma_start(out=itall[:], in_=idx[:, :])
            for _rep in range(reps):
                for q in range(nchunk):
                    # tbufs=1 pool: WAR dep guarantees at most one indirect
                    # DMA in flight (two concurrent ones corrupt offsets).
                    t = tpool.tile([128, 2 * wpp * E], _DT, tag="t")
                    nc.gpsimd.indirect_dma_start(
                        # 3D out AP: one offset consumed per (p, c) row slot.
                        # A flat [128, 2*wpp*E] out makes the DGE take ONE
                        # offset per partition and fetch consecutive rows
                        # (HW-probed failure mode).
                        out=t[:].rearrange("p (c e) -> p c e", e=E),
                        out_offset=None,
                        in_=x[:, :],
                        in_offset=bass.IndirectOffsetOnAxis(
                            ap=itall[:, 2 * wpp * q : 2 * wpp * (q + 1)],
                            axis=0,
                        ),
                    )
                    c = pool.tile([128, wpp * E], _DT, tag="c")
                    nc.vector.tensor_add(
                        out=c[:], in0=t[:, : wpp * E], in1=t[:, wpp * E :]
                    )
                    if not hscale:
                        nc.scalar.mul(c[:], c[:], 0.5)
                    nc.sync.dma_start(
                        out=y[q * cpw : (q + 1) * cpw, :].rearrange(
                            "(p j) e -> p (j e)", p=128
                        ),
                        in_=c[:],
                    )
    nc.compile()
    return nc


def _build_nc_wind(reps, bufs, dt, nchunk, tbufs=1):
    """Native indirect gather of 2-row windows (one InstDMACopy per chunk,
    offsets in raw elements via a 1-D source view; coef=1), then the wh-style
    fused STT combine (x host-scaled by 1/m, beta kills junk rows)."""
    _DT = _F32 if dt == "f32" else _BF16
    cpw = NWP // nchunk
    wpp = cpw // 128
    nc = bacc.Bacc(
        "TRN2", target_bir_lowering=False, debug=False, num_devices=NCORES
    )
    x = nc.dram_tensor("x", [ROWS, E], _DT, kind="ExternalInput").ap()
    idx = nc.dram_tensor("idx", [128, wpp * nchunk], mybir.dt.int32,
                         kind="ExternalInput").ap()
    bt = nc.dram_tensor("bt", [nchunk, 128, wpp], _DT,
                        kind="ExternalInput").ap()
    y = nc.dram_tensor("y", [NWP, E], _DT, kind="ExternalOutput").ap()
    with tile.TileContext(nc) as tc:
        with (
            tc.tile_pool(name="idxp", bufs=1) as ipool,
            tc.tile_pool(name="tp", bufs=tbufs) as tpool,
            tc.tile_pool(name="io", bufs=bufs) as pool,
        ):
            itall = ipool.tile([128, wpp * nchunk], mybir.dt.int32,
                               tag="itall")
            nc.sync.dma_start(out=itall[:], in_=idx[:, :])
            btall = ipool.tile([128, nchunk * wpp], _DT, tag="btall")
            nc.sync.dma_start(
                out=btall[:].rearrange("p (q s) -> p q s", q=nchunk),
                in_=bt.rearrange("q p s -> p q s"),
            )
            for _rep in range(reps):
                for q in range(nchunk):
                    t = tpool.tile([128, 2 * wpp * E], _DT, tag="t")
                    # coef = E (row stride); each (p, c) out slot pulls 2E
                    # contiguous elements = the 2-row window [s, s+2)
                    nc.gpsimd.indirect_dma_start(
                        out=t[:].rearrange("p (c e) -> p c e", e=2 * E),
                        out_offset=None,
                        in_=x[:, :],
                        in_offset=bass.IndirectOffsetOnAxis(
                            ap=itall[:, wpp * q : wpp * (q + 1)], axis=0),
                    )
                    c = pool.tile([128, wpp * E], _DT, tag="c")
                    for j in range(wpp):
                        nc.vector.scalar_tensor_tensor(
                            out=c[:, j * E : (j + 1) * E],
                            in0=t[:, (2 * j + 1) * E : (2 * j + 2) * E],
                            scalar=btall[:, q * wpp + j : q * wpp + j + 1],
                            in1=t[:, 2 * j * E : (2 * j + 1) * E],
                            op0=mybir.AluOpType.mult,
                            op1=mybir.AluOpType.add,
                        )
                    nc.sync.dma_start(
                        out=y[q * cpw : (q + 1) * cpw, :].rearrange(
                            "(p j) e -> p (j e)", p=128),
                        in_=c[:],
                    )
    nc.compile()
    return nc


def _build_nc_mm(reps=1, xbufs=1, pbufs=4, obufs=6):
    """Segment-sum as matmul: NO gathers, NO Q7 ucode (dodges the ~16us
    ucode-ready gate). Contiguous HWDGE loads of x; the 0/1 segment matrix
    M^T[t, w] = [s_w <= t] - [s_{w+1} <= t] is built on-chip from an
    uploaded (replicated) boundary row via two DVE ops per k-tile; TensorE
    computes out = M^T.T @ x per (sentence, word-tile); PSUM is evicted with
    a per-partition 1/m scale (the span mean) straight to bf16 and stored.

    Word layout: y row = sent*256 + w (pad word 255 per sentence interleaved).
    """
    NS = BPC                      # sentences per core = 8
    KT = S // 128                 # k-tiles per sentence = 4
    WT = 2                        # word-tiles per sentence (256 words)
    SW = 257                      # boundary cols per sentence (s_0..s_255, sentinel)
    nc = bacc.Bacc(
        "TRN2", target_bir_lowering=False, debug=False, num_devices=NCORES
    )
    x = nc.dram_tensor("x", [ROWS, E], _BF16, kind="ExternalInput").ap()
    sth = nc.dram_tensor("sth", [1, NS * SW], mybir.dt.float16,
                         kind="ExternalInput").ap()
    gcol = nc.dram_tensor("gcol", [128, KT], _F32, kind="ExternalInput").ap()
    msc = nc.dram_tensor("msc", [128, NS * WT], _F32,
                         kind="ExternalInput").ap()
    y = nc.dram_tensor("y", [NS * 256, E], _BF16, kind="ExternalOutput").ap()

    with tile.TileContext(nc) as tc:
        with (
            tc.tile_pool(name="const", bufs=1) as cpool,
            tc.tile_pool(name="xs", bufs=xbufs) as xpool,
            tc.tile_pool(name="m", bufs=1) as mpool,
            tc.tile_pool(name="ps", bufs=pbufs, space="PSUM") as ppool,
            tc.tile_pool(name="out", bufs=obufs) as opool,
        ):
            # x loads split across the SWDGE ring (gpsimd) and the ACT
            # engine's own HWDGE ring (qActDynamicHW) -- two descriptor
            # paths inject in parallel; the sync ring stays FIFO-clean for
            # the stores (mixing big loads with stores on one ring
            # head-of-line-blocks them; HW-measured +14us)
            # boundary row: 4KB flat upload + PE broadcast to 128 partitions
            # (a [128, 2056] replicated upload costs 0.53MB of HBM stream;
            # ones-matmul replication costs ~0)
            sfl = cpool.tile([1, NS * SW], mybir.dt.float16, tag="sfl")
            nc.sync.dma_start(out=sfl[:], in_=sth[:, :])
            ones = cpool.tile([1, 128], mybir.dt.float16, tag="ones")
            nc.vector.memset(ones[:], 1.0)
            stht = cpool.tile([128, NS * SW], mybir.dt.float16, tag="sth")
            off = 0
            while off < NS * SW:
                n = min(384, NS * SW - off)
                pb = ppool.tile([128, 384], _F32, tag="p0")
                nc.tensor.matmul(
                    pb[:, :n], ones[:], sfl[:, off : off + n],
                    start=True, stop=True,
                )
                nc.vector.tensor_copy(out=stht[:, off : off + n], in_=pb[:, :n])
                off += n
            gct = cpool.tile([128, KT], _F32, tag="gcol")
            nc.sync.dma_start(out=gct[:], in_=gcol[:, :])
            msct = cpool.tile([128, NS * WT], _F32, tag="msc")
            nc.sync.dma_start(out=msct[:], in_=msc[:, :])
            # last sentence per ring (s6 scalar, s7 gpsimd) split (k0-2)+(k3):
            # its wt0 matmuls overlap the final k3 transfer, shortening the
            # post-last-load chain
            xts = []
            xbs = {}
            for s in range(NS):
                eng = nc.scalar if s % 2 == 0 else nc.gpsimd
                xin = x[s * S : (s + 1) * S, :].rearrange(
                    "(k p) e -> p k e", p=128
                )
                if s >= NS - 2:
                    xa = xpool.tile([128, 3 * E], _BF16, tag=f"xa{s}")
                    eng.dma_start(
                        out=xa[:].rearrange("p (k e) -> p k e", e=E),
                        in_=x[s * S : s * S + 384, :].rearrange(
                            "(k p) e -> p k e", p=128
                        ),
                    )
                    xb = xpool.tile([128, E], _BF16, tag=f"xb{s}")
                    eng.dma_start(out=xb[:], in_=x[s * S + 384 : (s + 1) * S, :])
                    xts.append(xa)
                    xbs[s] = xb
                else:
                    xt = xpool.tile([128, KT * E], _BF16, tag=f"x{s}")
                    eng.dma_start(
                        out=xt[:].rearrange("p (k e) -> p k e", e=E),
                        in_=xin,
                    )
                    xts.append(xt)
            for _rep in range(reps):
                # cmp_k[p, sent*SW + w] = [s_w <= p + 128k]  (0/1 bf16)
                # build order: each cmp immediately followed by its s0 sub,
                # so (s0, wt0)'s blocks are ready ~1us sooner and the first
                # matmul (hence the whole eviction chain) starts earlier
                cmps = []
                mts = {}

                def _sub(k, s):
                    mt = mpool.tile([128, 256], _BF16, tag=f"m{k}_{s}")
                    nc.vector.tensor_tensor(
                        out=mt[:],
                        in0=cmps[k][:, s * SW : s * SW + 256],
                        in1=cmps[k][:, s * SW + 1 : s * SW + 257],
                        op=mybir.AluOpType.subtract,
                    )
                    mts[(k, s)] = mt

                for k in range(KT):
                    ck = mpool.tile([128, NS * SW], _BF16, tag=f"cmp{k}")
                    nc.vector.tensor_scalar(
                        ck[:], stht[:], gct[:, k : k + 1], None,
                        op0=mybir.AluOpType.is_le,
                    )
                    cmps.append(ck)
                    _sub(k, 0)
                for s in range(1, NS):
                    for k in range(KT):
                        _sub(k, s)
                # per (sent, wt): 3 k-blocks x 2 N-halves accumulate in PSUM
                for s in range(NS):
                    # one merged [128, 2E] out tile per sentence -> ONE store
                    # (8 stores instead of 16: half the issue/receipt cost)
                    ot = opool.tile([128, 2 * E], _BF16, tag="o")
                    for wt in range(WT):
                        ks = (0, 1, 2) if wt == 0 else (1, 2, 3)
                        pts = []
                        for h in range(2):
                            pt = ppool.tile([128, 384], _F32, tag=f"p{h}")
                            for i, k in enumerate(ks):
                                if k == 3 and s in xbs:
                                    rhs = xbs[s][:, h * 384 : (h + 1) * 384]
                                else:
                                    rhs = xts[s][:, k * E + h * 384 :
                                                 k * E + (h + 1) * 384]
                                nc.tensor.matmul(
                                    pt[:],
                                    mts[(k, s)][:, wt * 128 : (wt + 1) * 128],
                                    rhs,
                                    start=(i == 0),
                                    stop=(i == len(ks) - 1),
                                )
                            pts.append(pt)
                        col = s * WT + wt
                        # PSUM eviction + 1/m scale split across DVE and ACT
                        # (a single engine's 32-op eviction chain paces the
                        # whole tail at ~20us; HW-traced)
                        nc.vector.tensor_scalar_mul(
                            ot[:, wt * E : wt * E + 384],
                            pts[0][:], msct[:, col : col + 1]
                        )
                        nc.scalar.mul(
                            ot[:, wt * E + 384 : (wt + 1) * E],
                            pts[1][:], msct[:, col : col + 1]
                        )
                    nc.sync.dma_start(
                        out=y[s * 256 : (s + 1) * 256, :].rearrange(
                            "(c p) e -> p c e", p=128
                        ),
                        in_=ot[:].rearrange("p (c e) -> p c e", e=E),
                    )
    nc.compile()
    return nc


def _build_nc_acc(reps, bufs, dt="f32", nchunk=4):
    """Native indirect gathers, zero compute engines: host uploads x/2, the
    B gather CCE-accumulates onto the A gather in SBUF, and the only
    consumer is a DMA store (the HW-verified-safe consumer)."""
    _DT = _F32 if dt == "f32" else _BF16
    cpw = NWP // nchunk
    wpp = cpw // 128
    nc = bacc.Bacc(
        "TRN2", target_bir_lowering=False, debug=False, num_devices=NCORES
    )
    x = nc.dram_tensor("x", [ROWS, E], _DT, kind="ExternalInput").ap()
    idx = nc.dram_tensor("idx", [128, 2 * wpp * nchunk], mybir.dt.int32,
                         kind="ExternalInput").ap()
    y = nc.dram_tensor("y", [NWP, E], _DT, kind="ExternalOutput").ap()
    with tile.TileContext(nc) as tc:
        with (
            tc.tile_pool(name="idxp", bufs=1) as ipool,
            tc.tile_pool(name="io", bufs=bufs) as pool,
        ):
            itall = ipool.tile([128, 2 * wpp * nchunk], mybir.dt.int32,
                               tag="itall")
            nc.sync.dma_start(out=itall[:], in_=idx[:, :])
            for _rep in range(reps):
                for q in range(nchunk):
                    o = 2 * wpp * q
                    t = pool.tile([128, wpp * E], _DT, tag="t")
                    t3 = t[:].rearrange("p (c e) -> p c e", e=E)
                    nc.gpsimd.indirect_dma_start(
                        out=t3, out_offset=None, in_=x[:, :],
                        in_offset=bass.IndirectOffsetOnAxis(
                            ap=itall[:, o : o + wpp], axis=0),
                    )
                    nc.gpsimd.indirect_dma_start(
                        out=t3, out_offset=None, in_=x[:, :],
                        in_offset=bass.IndirectOffsetOnAxis(
                            ap=itall[:, o + wpp : o + 2 * wpp], axis=0),
                        compute_op=mybir.AluOpType.add,
                    )
                    nc.sync.dma_start(
                        out=y[q * cpw : (q + 1) * cpw, :].rearrange(
                            "(p j) e -> p (j e)", p=128),
                        in_=t[:],
                    )
    nc.compile()
    return nc


def _build_nc_sized(reps, bufs, sizes, hscale):
    """ab-mode kernel with per-chunk word counts `sizes` (multiples of 128
    summing to NWP). Small leading chunks start transfers sooner; small
    trailing chunks shorten the exposed compute+store tail."""
    assert sum(sizes) == NWP and all(s % 128 == 0 for s in sizes)
    nc = bacc.Bacc(
        "TRN2", target_bir_lowering=False, debug=False, num_devices=NCORES
    )
    x = nc.dram_tensor("x", [ROWS, E], _F32, kind="ExternalInput").ap()
    total_cols = sum(2 * s // 16 for s in sizes)
    idx = nc.dram_tensor("idx", [128, total_cols], _I16, kind="ExternalInput").ap()
    y = nc.dram_tensor("y", [NWP, E], _F32, kind="ExternalOutput").ap()
    jmax = max(sizes) // 128

    with tile.TileContext(nc) as tc:
        with (
            tc.tile_pool(name="idxp", bufs=1) as ipool,
            tc.tile_pool(name="io", bufs=bufs) as pool,
        ):
            itall = ipool.tile([128, total_cols], _I16, tag="itall")
            nc.sync.dma_start(out=itall[:], in_=idx[:, :])
            for _rep in range(reps):
                off_c = 0
                off_w = 0
                for s in sizes:
                    jj = s // 128
                    cols = 2 * s // 16
                    t = pool.tile([128, 2 * jmax * E], _F32, tag="t")
                    nc.gpsimd.dma_gather(
                        t[:, : 2 * jj * E].rearrange("p (c e) -> p c e", e=E),
                        x[:, :],
                        itall[:, off_c : off_c + cols],
                        2 * s,
                        2 * s,
                        E,
                    )
                    c = pool.tile([128, jmax * E], _F32, tag="c")
                    nc.vector.tensor_add(
                        out=c[:, : jj * E],
                        in0=t[:, : jj * E],
                        in1=t[:, jj * E : 2 * jj * E],
                    )
                    if not hscale:
                        nc.scalar.mul(c[:, : jj * E], c[:, : jj * E], 0.5)
                    nc.sync.dma_start(
                        out=y[off_w : off_w + s, :].rearrange(
                            "(p j) e -> p (j e)", p=128
                        ),
                        in_=c[:, : jj * E],
                    )
                    off_c += cols
                    off_w += s
    nc.compile()
    return nc


def _build_nc_jm(tmaxs, reps=1, warmups=12, pbufs=2, obufs=6, mbufs=3):
    """JIT-specialized segment-sum matmul (mode "jm", rev 2).

    tmaxs[s] = max total BPE rows over the 8 cores' sentences assigned to
    slot s (host sorts sentences by row count and deals groups of 8 across
    cores, so the envelope is tight).  HW-trace-driven structure:

    * Loads: uniform 3-tile (384-row) sentence loads -- slots with
      Tmax<384 over-read a few junk rows (weight 0 via the sentinel
      compare) so full-part loads merge into ONE 4D-AP DMA per sentence
      group; 5 DMA configs per ring (config costs ~1.3us on ACT, the
      rev-1 killer).  >384-row tails (a few rows) load on the sync ring.
    * Boundaries upload PRE-REPLICATED [128, 2056] fp16 (0.53MB): no PE
      ones-matmul replication, no DVE casts; PE does only real matmuls.
    * M^T blocks: per sentence TWO broadcast tensor_tensor compares (DVE)
      + ONE batched adjacent-diff (Pool) -- DVE op fixed cost ~300ns
      makes per-block compares (rev 1: 34 ops) a bottleneck.
    * PE p-state: ~12 dependency-free warmup matmuls fill the pre-load
      window so real matmuls start at the ramped ~325ns/384cols rate.
    * Evictions split DVE (h0) / ACT (h1); stores split per word-tile.
    """
    NS = BPC
    SW = 257
    nc = bacc.Bacc(
        "TRN2", target_bir_lowering=False, debug=False, num_devices=NCORES
    )
    x = nc.dram_tensor("x", [ROWS, E], _BF16, kind="ExternalInput").ap()
    sthr = nc.dram_tensor("sthr", [128, NS * SW], mybir.dt.float16,
                          kind="ExternalInput").ap()
    gcth = nc.dram_tensor("gcth", [128, 4], mybir.dt.float16,
                          kind="ExternalInput").ap()
    aux = nc.dram_tensor("aux", [128, NS * 2], _F32, kind="ExternalInput").ap()
    y = nc.dram_tensor("y", [NS * 256, E], _BF16, kind="ExternalOutput").ap()

    FT = 3                        # uniform full k-tiles per sentence
    geo = []
    for s in range(NS):
        T = tmaxs[s]
        assert 256 < T <= 512
        rem4 = max(0, T - 128 * FT)
        pairs = [(0, 0), (1, 0), (1, 1), (2, 1)] + ([(3, 1)] if rem4 else [])
        geo.append((T, rem4, pairs))
    nbmax = max(len(g[2]) for g in geo)

    with tile.TileContext(nc) as tc:
        with (
            tc.tile_pool(name="const", bufs=1) as cpool,
            tc.tile_pool(name="xs", bufs=1) as xpool,
            tc.tile_pool(name="m", bufs=mbufs) as mpool,
            tc.tile_pool(name="ps", bufs=pbufs, space="PSUM") as ppool,
            tc.tile_pool(name="out", bufs=obufs) as opool,
        ):
            # ---- x full-part loads: 2 rings, merged 4D-AP groups ----
            xts = [None] * NS
            for ring, slots in ((nc.scalar, (0, 2, 4, 6)),
                                (nc.gpsimd, (1, 3, 5, 7))):
                for s in slots:
                    kt_n = FT + (1 if geo[s][1] else 0)
                    xts[s] = xpool.tile([128, kt_n * E], _BF16, tag=f"x{s}",
                                        name=f"x{s}")
                # first slot per-ktile: unblocks the first matmuls fast
                s0 = slots[0]
                for kt in range(FT):
                    base = s0 * S + 1 + 128 * kt
                    ring.dma_start(out=xts[s0][:, kt * E : (kt + 1) * E],
                                   in_=x[base : base + 128, :])
                # remaining slots: one merged full-part DMA per sentence
                for sx in slots[1:]:
                    base = sx * S + 1
                    ring.dma_start(
                        out=xts[sx][:, : FT * E].rearrange(
                            "p (k e) -> p k e", e=E),
                        in_=x[base : base + 128 * FT, :].rearrange(
                            "(k p) e -> p k e", p=128))
            # ---- tiny uploads on sync (then tails, then stores) ----
            stht = cpool.tile([128, NS * SW], mybir.dt.float16, tag="stht")
            nc.sync.dma_start(out=stht[:], in_=sthr[:, :])
            gt = cpool.tile([128, 4], mybir.dt.float16, tag="gt")
            nc.sync.dma_start(out=gt[:], in_=gcth[:, :])
            auxt = cpool.tile([128, NS * 2], _F32, tag="auxt")
            nc.sync.dma_start(out=auxt[:], in_=aux[:, :])
            for s in range(NS):
                T, rem4, pairs = geo[s]
                if rem4:
                    xt, off = _xtof(xts[s])
                    base = s * S + 1 + 128 * FT
                    nc.sync.dma_start(out=xt[:rem4, off + FT * E :
                                             off + (FT + 1) * E],
                                      in_=x[base : base + rem4, :])
            # ---- PE p-state warmup: dependency-free matmuls ----
            wones = cpool.tile([1, 384], mybir.dt.float16, tag="wones")
            nc.vector.memset(wones[:], 1.0)
            for i in range(warmups):
                wpt = ppool.tile([128, 384], _F32, tag=f"p{i % 2}0",
                                 name="wpt")
                nc.tensor.matmul(wpt[:], wones[:, :128], wones[:],
                                 start=True, stop=True)
            for _rep in range(reps):
                for s in range(NS):
                    T, rem4, pairs = geo[s]
                    nb = len(pairs)
                    xt, off = _xtof(xts[s])
                    ct = mpool.tile([128, nbmax * 129], _BF16, tag="ct")
                    # wt0: kt 0,1 ; wt1: kt 1..  (broadcast compares)
                    b0 = stht[:, s * SW : s * SW + 129]
                    nc.vector.tensor_tensor(
                        out=ct[:, : 2 * 129].rearrange("p (b c) -> p b c",
                                                       c=129),
                        in0=b0.rearrange("p (b c) -> p b c", c=129)
                              .to_broadcast([128, 2, 129]),
                        in1=gt[:, 0:2].rearrange("p (b c) -> p b c", c=1)
                              .to_broadcast([128, 2, 129]),
                        op=mybir.AluOpType.is_le)
                    nk1 = nb - 2
                    b1 = stht[:, s * SW + 128 : s * SW + 257]
                    nc.vector.tensor_tensor(
                        out=ct[:, 2 * 129 : nb * 129].rearrange(
                            "p (b c) -> p b c", c=129),
                        in0=b1.rearrange("p (b c) -> p b c", c=129)
                              .to_broadcast([128, nk1, 129]),
                        in1=gt[:, 1 : 1 + nk1].rearrange("p (b c) -> p b c",
                                                         c=1)
                              .to_broadcast([128, nk1, 129]),
                        op=mybir.AluOpType.is_le)
                    mts = mpool.tile([128, nbmax * 128], _BF16, tag="mt")
                    ctv = ct[:, : nb * 129].rearrange("p (b c) -> p b c",
                                                      c=129)
                    mtv = mts[:, : nb * 128].rearrange("p (b c) -> p b c",
                                                       c=128)
                    nc.gpsimd.tensor_tensor(out=mtv, in0=ctv[:, :, 0:128],
                                            in1=ctv[:, :, 1:129],
                                            op=mybir.AluOpType.subtract)
                    for wt in range(2):
                        wps = [(bi, kt) for bi, (kt, w) in enumerate(pairs)
                               if w == wt]
                        pts = [ppool.tile([128, 384], _F32, tag=f"p{wt}{h}",
                                          name=f"pt{wt}{h}")
                               for h in range(2)]
                        for i, (bi, kt) in enumerate(wps):
                            wk = rem4 if kt == FT else 128
                            for h in range(2):
                                nc.tensor.matmul(
                                    pts[h][:],
                                    mts[:wk, bi * 128 : (bi + 1) * 128],
                                    xt[:wk, off + kt * E + h * 384
                                       : off + kt * E + h * 384 + 384],
                                    start=(i == 0), stop=(i == len(wps) - 1),
                                )
                        col = 2 * s + wt
                        ot = opool.tile([128, E], _BF16, tag="o", name="ot")
                        nc.vector.tensor_scalar_mul(
                            ot[:, :384], pts[0][:], auxt[:, col : col + 1])
                        nc.scalar.mul(
                            ot[:, 384:], pts[1][:], auxt[:, col : col + 1])
                        nc.sync.dma_start(
                            out=y[s * 256 + wt * 128
                                  : s * 256 + wt * 128 + 128, :],
                            in_=ot[:])
    nc.compile()
    return nc


def _xtof(entry):
    """xts[] entry -> (tile, col offset) for merged group tiles."""
    if isinstance(entry, tuple):
        return entry
    return entry, 0


def _make_in_maps_jm(output, mappings):
    """Host prep for mode "jm": sort sentences by total BPE rows, deal
    groups of 8 across cores (slot s, core k <- sentence order[s*8+k]);
    upload bf16 x in slot order, pre-replicated fp16 boundary rows,
    fp16 row-index columns, f32 1/m scales."""
    import ml_dtypes

    NS = BPC
    SW = 257
    output = np.asarray(output)
    mappings = np.asarray(mappings, dtype=np.int32)
    ends = np.cumsum(mappings, axis=1, dtype=np.int32)      # [B, W]
    Ti = ends[:, -1]                                        # [B]
    order = np.argsort(Ti, kind="stable")
    assign = order.reshape(NS, NCORES)                      # [slot, core]
    tmaxs = tuple(int(Ti[assign[s]].max()) for s in range(NS))
    xbf = output.astype(ml_dtypes.bfloat16)
    gcth = np.ascontiguousarray(
        (np.arange(128, dtype=np.float32)[:, None]
         + 128.0 * np.arange(4, dtype=np.float32)[None, :]).astype(np.float16))
    minv = 1.0 / mappings.astype(np.float32)
    in_maps = []
    for k in range(NCORES):
        sents = assign[:, k]
        xk = np.ascontiguousarray(xbf[sents].reshape(ROWS, E))
        sthk = np.zeros((NS, SW), np.float16)
        sthk[:, 1:256] = ends[sents]
        sthk[:, 256] = Ti[sents]
        sthr = np.ascontiguousarray(
            np.broadcast_to(sthk.reshape(1, -1), (128, NS * SW)))
        auxk = np.ones((128, NS * 2), np.float32)
        mi = minv[sents]
        for s in range(NS):
            auxk[:, 2 * s] = mi[s, 0:128]
            auxk[:127, 2 * s + 1] = mi[s, 128:255]
        in_maps.append({
            "x": xk,
            "sthr": sthr,
            "gcth": gcth,
            "aux": np.ascontiguousarray(auxk),
        })
    return in_maps, assign, tmaxs


def _run_jm(output, mappings, reps=1, warmups=12, pbufs=2, obufs=6,
            mbufs=3, **kw):
    in_maps, assign, tmaxs = _make_in_maps_jm(output, mappings)
    key = ("jm2", tmaxs, reps, warmups, pbufs, obufs, mbufs)
    if key not in _NC:
        _NC[key] = _build_nc_jm(tmaxs, reps, warmups, pbufs, obufs, mbufs)
    res = run_bass_kernel_spmd(_NC[key], in_maps, list(range(NCORES)), **kw)
    full = np.empty((B, W, E), np.float32)
    for k, r in enumerate(res.results):
        yk = np.asarray(r["y"], dtype=np.float32).reshape(BPC, 256, E)[:, :W]
        full[assign[:, k]] = yk
    return full, res


_NC = {}


def _get_nc(reps=1, bufs=2, order="pc", nq=1, mode="ab", nchunk=NCHUNK,
            merged_idx=False, warm=False, hscale=False, sizes=None,
            dt="f32", split0=False):
    key = (reps, bufs, order, nq, mode, nchunk, merged_idx, warm, hscale,
           tuple(sizes) if sizes else None, dt, split0)
    if key not in _NC:
        _NC[key] = _build_nc(reps, bufs, order, nq, mode, nchunk, merged_idx,
                             warm, hscale, sizes, dt, split0)
    return _NC[key]


def _wrap16(flat):
    """int16 index list -> [128, n/16] wrapped (i -> [i%16, i//16]) + 8x rep."""
    return np.tile(flat.reshape(-1, 16).T, (8, 1)).astype(np.int16)


def _make_in_maps(output, mappings, order="pc", mode="ab", nchunk=NCHUNK,
                  hscale=False, sizes=None, dt="f32"):
    output = np.asarray(output)
    if hscale:
        # fold the *0.5 of the span mean into the shard upload: a/2 + b/2
        # rounds identically to (a+b)/2 in f32 (halving is exact).
        output = output * np.float32(0.5)
    npdt = np.float32 if dt == "f32" else _np_bf16()
    mappings = np.asarray(mappings, dtype=np.int32)
    ends = np.cumsum(mappings, axis=1, dtype=np.int32)  # [B, W] exclusive ends
    src_a = ends - mappings + 1                         # +1: skip [CLS]
    src_b = ends                                        # (e-1) + 1
    if mode in ("wh", "wind"):
        # scale every BPE row by 1/m of its owning word (0.5/1 exact in f32,
        # single bf16 rounding after); junk rows keep scale 1 and are killed
        # on-device by beta=0.
        g = np.ones((B, S), np.float32)
        two = mappings == 2
        np.put_along_axis(
            g, src_a, np.where(two, np.float32(0.5), np.float32(1.0)), axis=1
        )
        bi, wi_ = np.nonzero(two)
        g[bi, src_a[bi, wi_] + 1] = 0.5
        output = output * g[:, :, None]
    output = np.ascontiguousarray(output.astype(npdt))

    if mode == "mm":
        in_maps = []
        gcol = (np.arange(128, dtype=np.float32)[:, None]
                + 128.0 * np.arange(4, dtype=np.float32)[None, :])
        gcol = np.ascontiguousarray(gcol)
        minv = 1.0 / mappings.astype(np.float32)          # [B, W]
        for k in range(NCORES):
            bs = slice(k * BPC, (k + 1) * BPC)
            sa = src_a[bs]                                # [8, 255] s-coords
            sth = np.empty((BPC, 257), np.float16)
            sth[:, :255] = sa
            sth[:, 255] = ends[bs, -1] + 1                # pad word start
            sth[:, 256] = 600.0                           # sentinel > 511
            sth = np.ascontiguousarray(sth.reshape(1, -1))  # [1, 2056]
            msc = np.ones((128, BPC * 2), np.float32)
            mi = minv[bs]                                 # [8, 255]
            for s_ in range(BPC):
                msc[:, s_ * 2] = mi[s_, 0:128]
                msc[:128 - 1, s_ * 2 + 1] = mi[s_, 128:255]
            xk = np.ascontiguousarray(
                output[bs].reshape(ROWS, E).astype(_np_bf16()))
            in_maps.append({"x": xk, "sth": sth, "gcol": gcol,
                            "msc": np.ascontiguousarray(msc)})
        return in_maps

    in_maps = []
    for k in range(NCORES):
        bs = slice(k * BPC, (k + 1) * BPC)
        base = (np.arange(BPC, dtype=np.int32) * S)[:, None]
        a = (src_a[bs] + base).reshape(-1)
        b = (src_b[bs] + base).reshape(-1)
        pad = np.zeros(NWP - NW, np.int32)
        a = np.concatenate([a, pad])  # [NWP] word-ordered flat row ids
        b = np.concatenate([b, pad])
        x = np.ascontiguousarray(output[bs].reshape(ROWS, E))
        if mode == "wind":
            wpp = NWP // nchunk // 128
            mm = np.concatenate(
                [mappings[bs].reshape(-1), np.ones(NWP - NW, np.int32)]
            )
            beta = (mm == 2).astype(npdt)
            # [p, q*wpp+j] = element offset of word q*cpw + p*wpp + j
            aw = a.reshape(nchunk, 128, wpp).transpose(1, 0, 2)  # [p, q, j]
            idx = np.ascontiguousarray(aw.reshape(128, -1).astype(np.int32))
            bt = np.empty((nchunk, 128, wpp), npdt)
            cpw_ = NWP // nchunk
            for q in range(nchunk):
                sl = slice(q * cpw_, (q + 1) * cpw_)
                bt[q] = beta[sl].reshape(128, wpp)
            in_maps.append({"x": x, "idx": idx, "bt": bt})
            continue
        if mode in ("ind", "acc"):
            nck = nchunk
            wpp = NWP // nck // 128
            ia = a.reshape(nck, 128, wpp).transpose(1, 0, 2)  # [p, q, j]
            ib = b.reshape(nck, 128, wpp).transpose(1, 0, 2)
            idx = np.concatenate(
                [np.concatenate([ia[:, q], ib[:, q]], axis=1)
                 for q in range(nck)],
                axis=1,
            ).astype(np.int32)  # [128, 2*wpp*nck], cols 2*wpp*q + j
            in_maps.append({"x": x, "idx": np.ascontiguousarray(idx)})
            continue
        if sizes is not None:
            segs = []
            off = 0
            for s in sizes:
                jj = s // 128
                aq = a[off : off + s].reshape(128, jj).T.ravel()
                bq = b[off : off + s].reshape(128, jj).T.ravel()
                segs.append(_wrap16(np.concatenate([aq, bq])))
                off += s
            in_maps.append({"x": x, "idx": np.concatenate(segs, axis=1)})
            continue
        cpw = NWP // nchunk
        jj = cpw // 128
        if mode == "ab":
            idx = np.empty((nchunk, 128, 2 * cpw // 16), np.int16)
            for q in range(nchunk):
                aq = a[q * cpw : (q + 1) * cpw]
                bq = b[q * cpw : (q + 1) * cpw]
                if order == "pc":
                    # gathered i = c*128 + p holds word q*cpw + p*jj + c
                    aq = aq.reshape(128, jj).T.ravel()
                    bq = bq.reshape(128, jj).T.ravel()
                # 'seq': gathered i holds word q*cpw + i (ascending rows)
                idx[q] = _wrap16(np.concatenate([aq, bq]))
            in_maps.append({"x": x, "idx": idx})
        elif mode == "wh":
            mm = np.concatenate(
                [mappings[bs].reshape(-1), np.ones(NWP - NW, np.int32)]
            )
            beta = (mm == 2).astype(npdt)
            idx = np.empty((nchunk, 128, cpw // 16), np.int16)
            bt = np.empty((nchunk, 128, jj), npdt)
            for q in range(nchunk):
                sl = slice(q * cpw, (q + 1) * cpw)
                idx[q] = _wrap16(a[sl].reshape(128, jj).T.ravel())
                bt[q] = beta[sl].reshape(128, jj)
            in_maps.append({"x": x, "idx": idx, "bt": bt})
        else:
            m = np.concatenate(
                [mappings[bs].reshape(-1), np.ones(NWP - NW, np.int32)]
            ).astype(np.float32)
            r1 = 1.0 / m
            r2 = (m - 1.0) / m
            rdt = np.float32 if mode == "ws" else npdt
            idx = np.empty((nchunk, 128, cpw // 16), np.int16)
            rw = np.empty((nchunk, 128, 2 * jj), rdt)
            for q in range(nchunk):
                sl = slice(q * cpw, (q + 1) * cpw)
                aq = a[sl].reshape(128, jj).T.ravel()  # i = c*128 + p
                idx[q] = _wrap16(aq)
                rw[q, :, 0::2] = r1[sl].reshape(128, jj).astype(rdt)
                rw[q, :, 1::2] = r2[sl].reshape(128, jj).astype(rdt)
            in_maps.append({"x": x, "idx": idx, "rw": rw})
    return in_maps


def _run(output, mappings, reps=1, bufs=2, order="pc", nq=1, mode="ab",
         nchunk=NCHUNK, merged_idx=False, warm=False, hscale=False,
         sizes=None, dt="f32", split0=False, split_first=2, pbufs=2,
         obufs=4, mbufs=3, **kw):
    if mode == "jm":
        return _run_jm(output, mappings, reps=reps, split_first=split_first,
                       pbufs=pbufs, obufs=obufs, mbufs=mbufs, **kw)
    in_maps = _make_in_maps(output, mappings, order, mode, nchunk, hscale,
                            sizes, dt)
    nc = _get_nc(reps, bufs, order, nq, mode, nchunk, merged_idx, warm,
                 hscale, sizes, dt, split0)
    res = run_bass_kernel_spmd(nc, in_maps, list(range(NCORES)), **kw)
    if mode == "mm":
        outs = [
            np.asarray(r["y"], dtype=np.float32).reshape(BPC, 256, E)[:, :W]
            for r in res.results
        ]
    else:
        outs = [
            np.asarray(r["y"][:NW], dtype=np.float32).reshape(BPC, W, E)
            for r in res.results
        ]
    return np.concatenate(outs, axis=0), res


# Best HW-verified configuration: JIT-specialized matmul segment-sum
# (mode "jm").  Prior best: mode "mm" @ ~42.4us median; runner-up kept
# working: dict(bufs=6, order="pc", nq=1, mode="wh", nchunk=8,
# merged_idx=True, split0=True, dt="bf16") @ ~52.7us.
_CFG = dict(mode="jm")


def kernel(output, mappings):
    full, _ = _run(output, mappings, **_CFG)
    return full



# revision 12
# speedup vs baseline: 1.1569x; 1.0099x over previous
"""BERT per-word mean-pool (segment reduce) on 8 Trainium2 NeuronCores.

Problem: output[B=64, S=512, E=768] f32, mappings[B, W=255] int32 (values 1 or 2).
Per sentence, strip [CLS]/[SEP], mean-pool contiguous BPE spans into word vectors.

Key identity: every word's span has 1 or 2 BPE rows.  With s = span start,
    out[w] = (1/m) * (hs rows s .. s+m-1 summed),  m in {1, 2}.

Sharding: pure data parallel, 8 sentences per core, no cross-core comms.
All device data in bf16 (tolerance 2e-2 >> bf16's ~2e-3); host casts.

Two competitive kernels, HW-measured:

* mode "mm" (default): segment-sum as matmul. NO gathers, NO Q7 ucode --
  dodges the ~16us ucode-ready gate that floors every dma_gather kernel.
  Contiguous loads of x (SWDGE+HWDGE alternating); the 0/1 segment matrix
  M^T[t, w] = [s_w <= t] - [s_{w+1} <= t] is built on-chip from an uploaded
  boundary row (2 DVE ops per 128-row k-tile); TensorE accumulates
  out = M^T.T @ x per (sentence, 128-word tile) in PSUM; ACT evicts with a
  per-partition 1/m scale; HWDGE stores.

* mode "wh": windowed ucode gather. One 2-row-window descriptor per word
  (InstDMAGatherAnt), x pre-scaled by 1/m on host, single fused DVE
  scalar_tensor_tensor per word-column: out = t1*[m==2] + t0.

Raw indirect InstDMACopy (modes ind/acc/wind) is broken on this runtime:
the DGE consumes ONE offset per partition of the out AP and fetches
consecutive rows after it (HW-probed); a 3D out AP makes it worse. Do not
use those modes.
"""

import numpy as np

from concourse import bacc, bass, mybir, tile
from concourse.bass_utils import run_bass_kernel_spmd

B, S, W, E = 64, 512, 255, 768
NCORES = 8
BPC = B // NCORES            # sentences per core
NW = BPC * W                 # 2040 real words per core
NWP = 2048                   # padded word count (multiple of 512)
NCHUNK = 4                   # chunks per core
CPW = NWP // NCHUNK          # 512 words per chunk
JJ = CPW // 128              # 4 words per partition per chunk
ROWS = BPC * S               # 4096 input rows per core
NIDX = 2 * CPW               # 1024 gather indices per chunk (A then B)

_F32 = mybir.dt.float32
_BF16 = mybir.dt.bfloat16
_I16 = mybir.dt.int16

_NPBF16 = None


def _np_bf16():
    global _NPBF16
    if _NPBF16 is None:
        import ml_dtypes

        _NPBF16 = ml_dtypes.bfloat16
    return _NPBF16


def _build_nc(reps=1, bufs=2, order="pc", nq=1, mode="ab", nchunk=NCHUNK,
              merged_idx=False, warm=False, hscale=False, sizes=None,
              dt="f32", split0=False):
    _DT = _F32 if dt == "f32" else _BF16
    if sizes is not None:
        return _build_nc_sized(reps, bufs, sizes, hscale)
    if mode == "ind":
        return _build_nc_ind(reps, bufs, dt, nchunk, hscale)
    if mode == "acc":
        return _build_nc_acc(reps, bufs, dt, nchunk)
    if mode == "wind":
        return _build_nc_wind(reps, bufs, dt, nchunk)
    if mode == "mm":
        return _build_nc_mm(reps)
    nc = bacc.Bacc(
        "TRN2",
        target_bir_lowering=False,
        debug=False,
        num_devices=NCORES,
        num_swdge_queues=nq,
    )
    x = nc.dram_tensor("x", [ROWS, E], _DT, kind="ExternalInput").ap()
    # indices are int16, wrapped [i%16, i//16] into 16 partitions and
    # replicated 8x down to 128 partitions (Q7 core replication).
    cpw = NWP // nchunk
    jj = cpw // 128
    nidx = 2 * cpw if mode == "ab" else cpw
    idx = nc.dram_tensor(
        "idx", [nchunk, 128, nidx // 16], _I16, kind="ExternalInput"
    ).ap()
    _RDT = _F32 if mode == "ws" else _DT  # tensor_scalar wants f32 scalars
    if mode in ("win", "ws"):
        # per word w: rw[p, 2c] = 1/m(w), rw[p, 2c+1] = (m(w)-1)/m(w)
        rw = nc.dram_tensor(
            "rw", [nchunk, 128, 2 * jj], _RDT, kind="ExternalInput"
        ).ap()
    if mode == "wh":
        # per word w: bt[p, c] = [m(w) == 2]; x rows pre-scaled by 1/m on host
        bt = nc.dram_tensor(
            "bt", [nchunk, 128, jj], _DT, kind="ExternalInput"
        ).ap()
    y = nc.dram_tensor("y", [NWP, E], _DT, kind="ExternalOutput").ap()

    with tile.TileContext(nc) as tc:
        with (
            tc.tile_pool(name="idxp", bufs=1) as ipool,
            tc.tile_pool(name="io", bufs=bufs) as pool,
        ):
            if warm:
                # dummy 16-index gather issued first: triggers the Q7
                # ucode IRAM fetch (~6us) while the idx loads stream in,
                # so the first real gather isn't stalled on it.
                wi = ipool.tile([128, 1], _I16, tag="warmi")
                nc.gpsimd.memset(wi[:], 0)
                wo = ipool.tile([128, E], _DT, tag="warmo")
                nc.gpsimd.dma_gather(
                    wo[:].rearrange("p (c e) -> p c e", e=E),
                    x[:, :], wi[:], 16, 16, E,
                )
            its, rts = [], []
            ncols = nidx // 16
            if merged_idx:
                its = []
                if split0:
                    # chunk-0 idx as its own tiny first DMA so the first
                    # gather isn't gated on the full idx upload
                    it0 = ipool.tile([128, ncols], _I16, tag="it0")
                    nc.sync.dma_start(out=it0[:], in_=idx[0])
                    its.append(it0[:])
                    itall = ipool.tile(
                        [128, (nchunk - 1) * ncols], _I16, tag="itall"
                    )
                    nc.sync.dma_start(
                        out=itall[:].rearrange(
                            "p (q s) -> p q s", q=nchunk - 1
                        ),
                        in_=idx[1:].rearrange("q p s -> p q s"),
                    )
                    its += [
                        itall[:, q * ncols : (q + 1) * ncols]
                        for q in range(nchunk - 1)
                    ]
                else:
                    itall = ipool.tile([128, nchunk * ncols], _I16, tag="itall")
                    nc.sync.dma_start(
                        out=itall[:].rearrange("p (q s) -> p q s", q=nchunk),
                        in_=idx.rearrange("q p s -> p q s"),
                    )
                    its = [
                        itall[:, q * ncols : (q + 1) * ncols]
                        for q in range(nchunk)
                    ]
            else:
                for q in range(nchunk):
                    it = ipool.tile([128, ncols], _I16, tag=f"it{q}")
                    nc.sync.dma_start(out=it[:], in_=idx[q])
                    its.append(it[:])
            if mode in ("win", "ws"):
                # single merged weight load (one HWDGE DMA for all chunks)
                rtall = ipool.tile([128, nchunk * 2 * jj], _RDT, tag="rtall")
                nc.sync.dma_start(
                    out=rtall[:].rearrange("p (q s) -> p q s", q=nchunk),
                    in_=rw.rearrange("q p s -> p q s"),
                )
                rts = [
                    rtall[:, q * 2 * jj : (q + 1) * 2 * jj]
                    for q in range(nchunk)
                ]
            if mode == "wh":
                btall = ipool.tile([128, nchunk * jj], _DT, tag="btall")
                nc.sync.dma_start(
                    out=btall[:].rearrange("p (q s) -> p q s", q=nchunk),
                    in_=bt.rearrange("q p s -> p q s"),
                )
            for _rep in range(reps):
                for q in range(nchunk):
                    if mode == "ab":
                        # gathered slot i -> T[i % 128, i // 128, :]
                        # i = c*128 + p:  c in 0..3 -> first-BPE row of word
                        # w = q*512 + p*4 + c;  c in 4..7 -> last-BPE row.
                        t = pool.tile([128, 2 * jj * E], _DT, tag="t")
                        nc.gpsimd.dma_gather(
                            t[:].rearrange("p (c e) -> p c e", e=E),
                            x[:, :],
                            its[q],
                            nidx,
                            nidx,
                            E,
                            queue_num=q % nq,
                        )
                        c = pool.tile([128, jj * E], _DT, tag="c")
                        nc.vector.tensor_add(
                            out=c[:], in0=t[:, : jj * E], in1=t[:, jj * E :]
                        )
                        if not hscale:
                            nc.scalar.mul(c[:], c[:], 0.5)
                    else:
                        # one 2-row window [s, s+2) per word, 6KB descriptors;
                        # out[w] = win[0]*r1 + win[1]*r2 kills the junk row
                        # (m=1: r=(1,0); m=2: r=(.5,.5)).
                        t = pool.tile([128, 2 * jj * E], _DT, tag="t")
                        xw = bass.AP(x.tensor, 0, [[E, ROWS - 1], [1, 2 * E]])
                        nc.gpsimd.dma_gather(
                            t[:].rearrange("p (c e) -> p c e", e=2 * E),
                            xw,
                            its[q],
                            cpw,
                            cpw,
                            2 * E,
                            elem_step=E,
                            queue_num=q % nq,
                        )
                        c = pool.tile([128, jj * E], _DT, tag="c")
                        if mode == "wh":
                            # x rows pre-scaled by 1/m on host; one fused DVE
                            # op per word-column: out = t1*[m==2] + t0, all
                            # APs contiguous [128, E] (full-rate DVE).
                            for j in range(jj):
                                nc.vector.scalar_tensor_tensor(
                                    out=c[:, j * E : (j + 1) * E],
                                    in0=t[:, (2 * j + 1) * E : (2 * j + 2) * E],
                                    scalar=btall[:, q * jj + j : q * jj + j + 1],
                                    in1=t[:, 2 * j * E : (2 * j + 1) * E],
                                    op0=mybir.AluOpType.mult,
                                    op1=mybir.AluOpType.add,
                                )
                        elif mode == "ws":
                            # device weights, contiguous slices: per word-col
                            # c3 = t1*r2; c = t0*r1 + c3
                            c3 = pool.tile([128, jj * E], _DT, tag="c3")
                            for j in range(jj):
                                nc.vector.tensor_scalar_mul(
                                    c3[:, j * E : (j + 1) * E],
                                    t[:, (2 * j + 1) * E : (2 * j + 2) * E],
                                    rts[q][:, 2 * j + 1 : 2 * j + 2],
                                )
                                nc.vector.scalar_tensor_tensor(
                                    out=c[:, j * E : (j + 1) * E],
                                    in0=t[:, 2 * j * E : (2 * j + 1) * E],
                                    scalar=rts[q][:, 2 * j : 2 * j + 1],
                                    in1=c3[:, j * E : (j + 1) * E],
                                    op0=mybir.AluOpType.mult,
                                    op1=mybir.AluOpType.add,
                                )
                        else:
                            t3 = t[:].rearrange("p (c e) -> p c e", e=2 * E)
                            r3 = rts[q].rearrange("p (c f) -> p c f", f=2)
                            c3 = pool.tile([128, jj * E], _DT, tag="c3")
                            cv = c[:].rearrange("p (j e) -> p j e", e=E)
                            c3v = c3[:].rearrange("p (j e) -> p j e", e=E)
                            nc.vector.tensor_tensor(
                                out=cv,
                                in0=t3[:, :, :E],
                                in1=r3[:, :, 0:1].to_broadcast([128, jj, E]),
                                op=mybir.AluOpType.mult,
                            )
                            nc.vector.tensor_tensor(
                                out=c3v,
                                in0=t3[:, :, E:],
                                in1=r3[:, :, 1:2].to_broadcast([128, jj, E]),
                                op=mybir.AluOpType.mult,
                            )
                            nc.vector.tensor_add(out=c[:], in0=c[:], in1=c3[:])
                    ychunk = y[q * cpw : (q + 1) * cpw, :]
                    if order == "pc":
                        nc.sync.dma_start(
                            out=ychunk.rearrange("(p j) e -> p (j e)", p=128),
                            in_=c[:],
                        )
                    else:
                        nc.sync.dma_start(
                            out=ychunk.rearrange("(j p) e -> p j e", p=128),
                            in_=c[:].rearrange("p (j e) -> p j e", e=E),
                        )
    nc.compile()
    return nc


def _build_nc_ind(reps, bufs, dt="f32", nchunk=4, hscale=False, tbufs=1):
    """Native SWDGE indirect gather (no ucode library -> no ~16us Q7 ucode
    ready gate). One InstDMACopy per chunk reading a column slice of a single
    int32 offset tile; slices advance monotonically (the HW-verified-safe
    pattern). Per chunk: first wpp cols = first-BPE rows, next wpp cols =
    last-BPE rows of words w = q*cpw + p*wpp + j."""
    _DT = _F32 if dt == "f32" else _BF16
    cpw = NWP // nchunk
    wpp = cpw // 128              # words per partition per chunk
    nc = bacc.Bacc(
        "TRN2", target_bir_lowering=False, debug=False, num_devices=NCORES
    )
    x = nc.dram_tensor("x", [ROWS, E], _DT, kind="ExternalInput").ap()
    idx = nc.dram_tensor("idx", [128, 2 * wpp * nchunk], mybir.dt.int32,
                         kind="ExternalInput").ap()
    y = nc.dram_tensor("y", [NWP, E], _DT, kind="ExternalOutput").ap()
    with tile.TileContext(nc) as tc:
        with (
            tc.tile_pool(name="idxp", bufs=1) as ipool,
            tc.tile_pool(name="tp", bufs=tbufs) as tpool,
            tc.tile_pool(name="io", bufs=bufs) as pool,
        ):
            itall = ipool.tile([128, 2 * wpp * nchunk], mybir.dt.int32,
                               tag="itall")
            nc.sync.dma_start(out=itall[:], in_=idx[:, :])
            for _rep in range(reps):
                for q in range(nchunk):
                    # tbufs=1 pool: WAR dep guarantees at most one indirect
                    # DMA in flight (two concurrent ones corrupt offsets).
                    t = tpool.tile([128, 2 * wpp * E], _DT, tag="t")
                    nc.gpsimd.indirect_dma_start(
                        # 3D out AP: one offset consumed per (p, c) row slot.
                        # A flat [128, 2*wpp*E] out makes the DGE take ONE
                        # offset per partition and fetch consecutive rows
                        # (HW-probed failure mode).
                        out=t[:].rearrange("p (c e) -> p c e", e=E),
                        out_offset=None,
                        in_=x[:, :],
                        in_offset=bass.IndirectOffsetOnAxis(
                            ap=itall[:, 2 * wpp * q : 2 * wpp * (q + 1)],
                            axis=0,
                        ),
                    )
                    c = pool.tile([128, wpp * E], _DT, tag="c")
                    nc.vector.tensor_add(
                        out=c[:], in0=t[:, : wpp * E], in1=t[:, wpp * E :]
                    )
                    if not hscale:
                        nc.scalar.mul(c[:], c[:], 0.5)
                    nc.sync.dma_start(
                        out=y[q * cpw : (q + 1) * cpw, :].rearrange(
                            "(p j) e -> p (j e)", p=128
                        ),
                        in_=c[:],
                    )
    nc.compile()
    return nc


def _build_nc_wind(reps, bufs, dt, nchunk, tbufs=1):
    """Native indirect gather of 2-row windows (one InstDMACopy per chunk,
    offsets in raw elements via a 1-D source view; coef=1), then the wh-style
    fused STT combine (x host-scaled by 1/m, beta kills junk rows)."""
    _DT = _F32 if dt == "f32" else _BF16
    cpw = NWP // nchunk
    wpp = cpw // 128
    nc = bacc.Bacc(
        "TRN2", target_bir_lowering=False, debug=False, num_devices=NCORES
    )
    x = nc.dram_tensor("x", [ROWS, E], _DT, kind="ExternalInput").ap()
    idx = nc.dram_tensor("idx", [128, wpp * nchunk], mybir.dt.int32,
                         kind="ExternalInput").ap()
    bt = nc.dram_tensor("bt", [nchunk, 128, wpp], _DT,
                        kind="ExternalInput").ap()
    y = nc.dram_tensor("y", [NWP, E], _DT, kind="ExternalOutput").ap()
    with tile.TileContext(nc) as tc:
        with (
            tc.tile_pool(name="idxp", bufs=1) as ipool,
            tc.tile_pool(name="tp", bufs=tbufs) as tpool,
            tc.tile_pool(name="io", bufs=bufs) as pool,
        ):
            itall = ipool.tile([128, wpp * nchunk], mybir.dt.int32,
                               tag="itall")
            nc.sync.dma_start(out=itall[:], in_=idx[:, :])
            btall = ipool.tile([128, nchunk * wpp], _DT, tag="btall")
            nc.sync.dma_start(
                out=btall[:].rearrange("p (q s) -> p q s", q=nchunk),
                in_=bt.rearrange("q p s -> p q s"),
            )
            for _rep in range(reps):
                for q in range(nchunk):
                    t = tpool.tile([128, 2 * wpp * E], _DT, tag="t")
                    # coef = E (row stride); each (p, c) out slot pulls 2E
                    # contiguous elements = the 2-row window [s, s+2)
                    nc.gpsimd.indirect_dma_start(
                        out=t[:].rearrange("p (c e) -> p c e", e=2 * E),
                        out_offset=None,
                        in_=x[:, :],
                        in_offset=bass.IndirectOffsetOnAxis(
                            ap=itall[:, wpp * q : wpp * (q + 1)], axis=0),
                    )
                    c = pool.tile([128, wpp * E], _DT, tag="c")
                    for j in range(wpp):
                        nc.vector.scalar_tensor_tensor(
                            out=c[:, j * E : (j + 1) * E],
                            in0=t[:, (2 * j + 1) * E : (2 * j + 2) * E],
                            scalar=btall[:, q * wpp + j : q * wpp + j + 1],
                            in1=t[:, 2 * j * E : (2 * j + 1) * E],
                            op0=mybir.AluOpType.mult,
                            op1=mybir.AluOpType.add,
                        )
                    nc.sync.dma_start(
                        out=y[q * cpw : (q + 1) * cpw, :].rearrange(
                            "(p j) e -> p (j e)", p=128),
                        in_=c[:],
                    )
    nc.compile()
    return nc


def _build_nc_mm(reps=1, xbufs=1, pbufs=4, obufs=6):
    """Segment-sum as matmul: NO gathers, NO Q7 ucode (dodges the ~16us
    ucode-ready gate). Contiguous HWDGE loads of x; the 0/1 segment matrix
    M^T[t, w] = [s_w <= t] - [s_{w+1} <= t] is built on-chip from an
    uploaded (replicated) boundary row via two DVE ops per k-tile; TensorE
    computes out = M^T.T @ x per (sentence, word-tile); PSUM is evicted with
    a per-partition 1/m scale (the span mean) straight to bf16 and stored.

    Word layout: y row = sent*256 + w (pad word 255 per sentence interleaved).
    """
    NS = BPC                      # sentences per core = 8
    KT = S // 128                 # k-tiles per sentence = 4
    WT = 2                        # word-tiles per sentence (256 words)
    SW = 257                      # boundary cols per sentence (s_0..s_255, sentinel)
    nc = bacc.Bacc(
        "TRN2", target_bir_lowering=False, debug=False, num_devices=NCORES
    )
    x = nc.dram_tensor("x", [ROWS, E], _BF16, kind="ExternalInput").ap()
    sth = nc.dram_tensor("sth", [1, NS * SW], mybir.dt.float16,
                         kind="ExternalInput").ap()
    gcol = nc.dram_tensor("gcol", [128, KT], _F32, kind="ExternalInput").ap()
    msc = nc.dram_tensor("msc", [128, NS * WT], _F32,
                         kind="ExternalInput").ap()
    y = nc.dram_tensor("y", [NS * 256, E], _BF16, kind="ExternalOutput").ap()

    with tile.TileContext(nc) as tc:
        with (
            tc.tile_pool(name="const", bufs=1) as cpool,
            tc.tile_pool(name="xs", bufs=xbufs) as xpool,
            tc.tile_pool(name="m", bufs=1) as mpool,
            tc.tile_pool(name="ps", bufs=pbufs, space="PSUM") as ppool,
            tc.tile_pool(name="out", bufs=obufs) as opool,
        ):
            # x loads split across the SWDGE ring (gpsimd) and the ACT
            # engine's own HWDGE ring (qActDynamicHW) -- two descriptor
            # paths inject in parallel; the sync ring stays FIFO-clean for
            # the stores (mixing big loads with stores on one ring
            # head-of-line-blocks them; HW-measured +14us)
            # boundary row: 4KB flat upload + PE broadcast to 128 partitions
            # (a [128, 2056] replicated upload costs 0.53MB of HBM stream;
            # ones-matmul replication costs ~0)
            sfl = cpool.tile([1, NS * SW], mybir.dt.float16, tag="sfl")
            nc.sync.dma_start(out=sfl[:], in_=sth[:, :])
            ones = cpool.tile([1, 128], mybir.dt.float16, tag="ones")
            nc.vector.memset(ones[:], 1.0)
            stht = cpool.tile([128, NS * SW], mybir.dt.float16, tag="sth")
            off = 0
            while off < NS * SW:
                n = min(384, NS * SW - off)
                pb = ppool.tile([128, 384], _F32, tag="p0")
                nc.tensor.matmul(
                    pb[:, :n], ones[:], sfl[:, off : off + n],
                    start=True, stop=True,
                )
                nc.vector.tensor_copy(out=stht[:, off : off + n], in_=pb[:, :n])
                off += n
            gct = cpool.tile([128, KT], _F32, tag="gcol")
            nc.sync.dma_start(out=gct[:], in_=gcol[:, :])
            msct = cpool.tile([128, NS * WT], _F32, tag="msc")
            nc.sync.dma_start(out=msct[:], in_=msc[:, :])
            # last sentence per ring (s6 scalar, s7 gpsimd) split (k0-2)+(k3):
            # its wt0 matmuls overlap the final k3 transfer, shortening the
            # post-last-load chain
            xts = []
            xbs = {}
            for s in range(NS):
                eng = nc.scalar if s % 2 == 0 else nc.gpsimd
                xin = x[s * S : (s + 1) * S, :].rearrange(
                    "(k p) e -> p k e", p=128
                )
                if s >= NS - 2:
                    xa = xpool.tile([128, 3 * E], _BF16, tag=f"xa{s}")
                    eng.dma_start(
                        out=xa[:].rearrange("p (k e) -> p k e", e=E),
                        in_=x[s * S : s * S + 384, :].rearrange(
                            "(k p) e -> p k e", p=128
                        ),
                    )
                    xb = xpool.tile([128, E], _BF16, tag=f"xb{s}")
                    eng.dma_start(out=xb[:], in_=x[s * S + 384 : (s + 1) * S, :])
                    xts.append(xa)
                    xbs[s] = xb
                else:
                    xt = xpool.tile([128, KT * E], _BF16, tag=f"x{s}")
                    eng.dma_start(
                        out=xt[:].rearrange("p (k e) -> p k e", e=E),
                        in_=xin,
                    )
                    xts.append(xt)
            for _rep in range(reps):
                # cmp_k[p, sent*SW + w] = [s_w <= p + 128k]  (0/1 bf16)
                # build order: each cmp immediately followed by its s0 sub,
                # so (s0, wt0)'s blocks are ready ~1us sooner and the first
                # matmul (hence the whole eviction chain) starts earlier
                cmps = []
                mts = {}

                def _sub(k, s):
                    mt = mpool.tile([128, 256], _BF16, tag=f"m{k}_{s}")
                    nc.vector.tensor_tensor(
                        out=mt[:],
                        in0=cmps[k][:, s * SW : s * SW + 256],
                        in1=cmps[k][:, s * SW + 1 : s * SW + 257],
                        op=mybir.AluOpType.subtract,
                    )
                    mts[(k, s)] = mt

                for k in range(KT):
                    ck = mpool.tile([128, NS * SW], _BF16, tag=f"cmp{k}")
                    nc.vector.tensor_scalar(
                        ck[:], stht[:], gct[:, k : k + 1], None,
                        op0=mybir.AluOpType.is_le,
                    )
                    cmps.append(ck)
                    _sub(k, 0)
                for s in range(1, NS):
                    for k in range(KT):
                        _sub(k, s)
                # per (sent, wt): 3 k-blocks x 2 N-halves accumulate in PSUM
                for s in range(NS):
                    # one merged [128, 2E] out tile per sentence -> ONE store
                    # (8 stores instead of 16: half the issue/receipt cost)
                    ot = opool.tile([128, 2 * E], _BF16, tag="o")
                    for wt in range(WT):
                        ks = (0, 1, 2) if wt == 0 else (1, 2, 3)
                        pts = []
                        for h in range(2):
                            pt = ppool.tile([128, 384], _F32, tag=f"p{h}")
                            for i, k in enumerate(ks):
                                if k == 3 and s in xbs:
                                    rhs = xbs[s][:, h * 384 : (h + 1) * 384]
                                else:
                                    rhs = xts[s][:, k * E + h * 384 :
                                                 k * E + (h + 1) * 384]
                                nc.tensor.matmul(
                                    pt[:],
                                    mts[(k, s)][:, wt * 128 : (wt + 1) * 128],
                                    rhs,
                                    start=(i == 0),
                                    stop=(i == len(ks) - 1),
                                )
                            pts.append(pt)
                        col = s * WT + wt
                        # PSUM eviction + 1/m scale split across DVE and ACT
                        # (a single engine's 32-op eviction chain paces the
                        # whole tail at ~20us; HW-traced)
                        nc.vector.tensor_scalar_mul(
                            ot[:, wt * E : wt * E + 384],
                            pts[0][:], msct[:, col : col + 1]
                        )
                        nc.scalar.mul(
                            ot[:, wt * E + 384 : (wt + 1) * E],
                            pts[1][:], msct[:, col : col + 1]
                        )
                    nc.sync.dma_start(
                        out=y[s * 256 : (s + 1) * 256, :].rearrange(
                            "(c p) e -> p c e", p=128
                        ),
                        in_=ot[:].rearrange("p (c e) -> p c e", e=E),
                    )
    nc.compile()
    return nc


def _build_nc_acc(reps, bufs, dt="f32", nchunk=4):
    """Native indirect gathers, zero compute engines: host uploads x/2, the
    B gather CCE-accumulates onto the A gather in SBUF, and the only
    consumer is a DMA store (the HW-verified-safe consumer)."""
    _DT = _F32 if dt == "f32" else _BF16
    cpw = NWP // nchunk
    wpp = cpw // 128
    nc = bacc.Bacc(
        "TRN2", target_bir_lowering=False, debug=False, num_devices=NCORES
    )
    x = nc.dram_tensor("x", [ROWS, E], _DT, kind="ExternalInput").ap()
    idx = nc.dram_tensor("idx", [128, 2 * wpp * nchunk], mybir.dt.int32,
                         kind="ExternalInput").ap()
    y = nc.dram_tensor("y", [NWP, E], _DT, kind="ExternalOutput").ap()
    with tile.TileContext(nc) as tc:
        with (
            tc.tile_pool(name="idxp", bufs=1) as ipool,
            tc.tile_pool(name="io", bufs=bufs) as pool,
        ):
            itall = ipool.tile([128, 2 * wpp * nchunk], mybir.dt.int32,
                               tag="itall")
            nc.sync.dma_start(out=itall[:], in_=idx[:, :])
            for _rep in range(reps):
                for q in range(nchunk):
                    o = 2 * wpp * q
                    t = pool.tile([128, wpp * E], _DT, tag="t")
                    t3 = t[:].rearrange("p (c e) -> p c e", e=E)
                    nc.gpsimd.indirect_dma_start(
                        out=t3, out_offset=None, in_=x[:, :],
                        in_offset=bass.IndirectOffsetOnAxis(
                            ap=itall[:, o : o + wpp], axis=0),
                    )
                    nc.gpsimd.indirect_dma_start(
                        out=t3, out_offset=None, in_=x[:, :],
                        in_offset=bass.IndirectOffsetOnAxis(
                            ap=itall[:, o + wpp : o + 2 * wpp], axis=0),
                        compute_op=mybir.AluOpType.add,
                    )
                    nc.sync.dma_start(
                        out=y[q * cpw : (q + 1) * cpw, :].rearrange(
                            "(p j) e -> p (j e)", p=128),
                        in_=t[:],
                    )
    nc.compile()
    return nc


def _build_nc_sized(reps, bufs, sizes, hscale):
    """ab-mode kernel with per-chunk word counts `sizes` (multiples of 128
    summing to NWP). Small leading chunks start transfers sooner; small
    trailing chunks shorten the exposed compute+store tail."""
    assert sum(sizes) == NWP and all(s % 128 == 0 for s in sizes)
    nc = bacc.Bacc(
        "TRN2", target_bir_lowering=False, debug=False, num_devices=NCORES
    )
    x = nc.dram_tensor("x", [ROWS, E], _F32, kind="ExternalInput").ap()
    total_cols = sum(2 * s // 16 for s in sizes)
    idx = nc.dram_tensor("idx", [128, total_cols], _I16, kind="ExternalInput").ap()
    y = nc.dram_tensor("y", [NWP, E], _F32, kind="ExternalOutput").ap()
    jmax = max(sizes) // 128

    with tile.TileContext(nc) as tc:
        with (
            tc.tile_pool(name="idxp", bufs=1) as ipool,
            tc.tile_pool(name="io", bufs=bufs) as pool,
        ):
            itall = ipool.tile([128, total_cols], _I16, tag="itall")
            nc.sync.dma_start(out=itall[:], in_=idx[:, :])
            for _rep in range(reps):
                off_c = 0
                off_w = 0
                for s in sizes:
                    jj = s // 128
                    cols = 2 * s // 16
                    t = pool.tile([128, 2 * jmax * E], _F32, tag="t")
                    nc.gpsimd.dma_gather(
                        t[:, : 2 * jj * E].rearrange("p (c e) -> p c e", e=E),
                        x[:, :],
                        itall[:, off_c : off_c + cols],
                        2 * s,
                        2 * s,
                        E,
                    )
                    c = pool.tile([128, jmax * E], _F32, tag="c")
                    nc.vector.tensor_add(
                        out=c[:, : jj * E],
                        in0=t[:, : jj * E],
                        in1=t[:, jj * E : 2 * jj * E],
                    )
                    if not hscale:
                        nc.scalar.mul(c[:, : jj * E], c[:, : jj * E], 0.5)
                    nc.sync.dma_start(
                        out=y[off_w : off_w + s, :].rearrange(
                            "(p j) e -> p (j e)", p=128
                        ),
                        in_=c[:, : jj * E],
                    )
                    off_c += cols
                    off_w += s
    nc.compile()
    return nc


def _build_nc_jm(tmaxs, reps=1, warmups=7, pbufs=2, obufs=6, mbufs=4):
    """JIT-specialized segment-sum matmul (mode "jm", rev 3).

    tmaxs[s] = max total BPE rows over the 8 cores' sentences assigned to
    slot s (host sorts sentences by row count and deals groups of 8 across
    cores).  HW-trace-driven rev-3 structure:

    * PE pace ramps ~320 -> ~185ns per matmul when fed continuously
      (p-state resets on any stall), so everything is organized to keep
      the matmul stream gap-free: k=128 warmup matmuls bridge from the
      post-prologue idle to the first real matmul; loads, M-builds and
      evictions are paced to stay a sentence ahead.
    * Loads: uniform 3-tile (384-row) merged loads, 4 configs on ACT ring
      (even slots) + 4 on SWDGE/Pool (odd slots); >384-row tails (a few
      rows) on sync.  DVE ring stays free for the M-build chain.
    * Boundaries upload pre-replicated fp16, split s0-s1 first so the DVE
      chain starts ~9.7us; M-builds per-sentence for s0/s1 then
      pair-batched (uniform nb=5 block tiles) to outpace the PE.
    * Matmul N-split 512+256 -> each (sentence, wordtile) PSUM is ONE
      [128,768] 2-bank tile, evicted+scaled by ONE op (ACT; DVE for the
      final sentence) and stored per wordtile.
    """
    NS = BPC
    SW = 257
    NB = 5                        # uniform M blocks per sentence
    nc = bacc.Bacc(
        "TRN2", target_bir_lowering=False, debug=False, num_devices=NCORES
    )
    x = nc.dram_tensor("x", [ROWS, E], _BF16, kind="ExternalInput").ap()
    sthr = nc.dram_tensor("sthr", [128, NS * SW], mybir.dt.float16,
                          kind="ExternalInput").ap()
    gcth = nc.dram_tensor("gcth", [128, 4], mybir.dt.float16,
                          kind="ExternalInput").ap()
    aux = nc.dram_tensor("aux", [128, NS * 2], _F32, kind="ExternalInput").ap()
    y = nc.dram_tensor("y", [NS * 256, E], _BF16, kind="ExternalOutput").ap()

    FT = 3                        # uniform full k-tiles per sentence
    geo = []
    for s in range(NS):
        T = tmaxs[s]
        assert 256 < T <= 512
        rem4 = max(0, T - 128 * FT)
        pairs = [(0, 0), (1, 0), (1, 1), (2, 1)] + ([(3, 1)] if rem4 else [])
        geo.append((T, rem4, pairs))

    with tile.TileContext(nc) as tc:
        with (
            tc.tile_pool(name="const", bufs=1) as cpool,
            tc.tile_pool(name="xs", bufs=1) as xpool,
            tc.tile_pool(name="m", bufs=mbufs) as mpool,
            tc.tile_pool(name="ps", bufs=pbufs, space="PSUM") as ppool,
            tc.tile_pool(name="out", bufs=obufs) as opool,
        ):
            # ---- x full-part loads: ACT ring (even) + SWDGE ring (odd) ----
            xts = [None] * NS
            for s in range(NS):
                kt_n = FT + (1 if geo[s][1] else 0)
                xts[s] = xpool.tile([128, kt_n * E], _BF16, tag=f"x{s}",
                                    name=f"x{s}")
            for ring, slots in ((nc.scalar, (0, 2, 4, 6)),
                                (nc.gpsimd, (1, 3, 5, 7))):
                for sx in slots:
                    base = sx * S + 1
                    ring.dma_start(
                        out=xts[sx][:, : FT * E].rearrange(
                            "p (k e) -> p k e", e=E),
                        in_=x[base : base + 128 * FT, :].rearrange(
                            "(k p) e -> p k e", p=128))
            # >384-row tails on the SWDGE ring after its merged loads
            for s in range(NS):
                T, rem4, pairs = geo[s]
                if rem4:
                    base = s * S + 1 + 128 * FT
                    nc.gpsimd.dma_start(
                        out=xts[s][:rem4, FT * E : (FT + 1) * E],
                        in_=x[base : base + rem4, :])
            # ---- uploads on sync: s0-s1 boundaries first ----
            stht = cpool.tile([128, NS * SW], mybir.dt.float16, tag="stht")
            nc.sync.dma_start(out=stht[:, : 2 * SW], in_=sthr[:, : 2 * SW])
            gt = cpool.tile([128, 4], mybir.dt.float16, tag="gt")
            nc.sync.dma_start(out=gt[:], in_=gcth[:, :])
            auxt = cpool.tile([128, NS * 2], _F32, tag="auxt")
            nc.sync.dma_start(out=auxt[:], in_=aux[:, :])
            nc.sync.dma_start(out=stht[:, 2 * SW :], in_=sthr[:, 2 * SW :])
            # ---- PE p-state warmup: dependency-free k=128 matmuls ----
            wt0 = cpool.tile([128, 512], _BF16, tag="wt0")
            nc.vector.memset(wt0[:], 1.0)
            for i in range(warmups):
                wpt = ppool.tile([128, 768], _F32, tag=f"p{i % 2}",
                                 name="wpt")
                nc.tensor.matmul(wpt[:, :512], wt0[:, :128], wt0[:],
                                 start=True, stop=True)

            def _mbuild(group):
                """Emit compares+diff for a group of adjacent slots; all
                ct/mts tiles use uniform NB blocks (junk blocks unused)."""
                g0, gn = group[0], len(group)
                ct = mpool.tile([128, gn * NB * 129], _BF16, tag=f"ct{gn}",
                                name="ct")
                ctv = ct[:].rearrange("p (s b c) -> p s b c", s=gn, c=129)
                s3 = stht[:, g0 * SW : (g0 + gn) * SW].rearrange(
                    "p (s c) -> p s c", c=SW)
                # wt0 blocks (b 0,1): kt 0,1
                nc.vector.tensor_tensor(
                    out=ctv[:, :, 0:2, :],
                    in0=s3[:, :, 0:129]
                        .rearrange("p s (b c) -> p s b c", b=1)
                        .to_broadcast([128, gn, 2, 129]),
                    in1=gt[:, 0:2]
                        .rearrange("p (s b c) -> p s b c", s=1, c=1)
                        .to_broadcast([128, gn, 2, 129]),
                    op=mybir.AluOpType.is_le)
                # wt1 blocks (b 2..4): kt 1..3
                nc.vector.tensor_tensor(
                    out=ctv[:, :, 2:NB, :],
                    in0=s3[:, :, 128:257]
                        .rearrange("p s (b c) -> p s b c", b=1)
                        .to_broadcast([128, gn, NB - 2, 129]),
                    in1=gt[:, 1 : NB - 1]
                        .rearrange("p (s b c) -> p s b c", s=1, c=1)
                        .to_broadcast([128, gn, NB - 2, 129]),
                    op=mybir.AluOpType.is_le)
                mts = mpool.tile([128, gn * NB * 128], _BF16, tag=f"mt{gn}",
                                 name="mts")
                cf = ct[:].rearrange("p (b c) -> p b c", c=129)
                mf = mts[:].rearrange("p (b c) -> p b c", c=128)
                nc.vector.tensor_tensor(out=mf, in0=cf[:, :, 0:128],
                                        in1=cf[:, :, 1:129],
                                        op=mybir.AluOpType.subtract)
                return mts

            groups = [(0,), (1,), (2, 3), (4, 5), (6, 7)]
            for _rep in range(reps):
                mtss = {}
                emitted = []
                gi = 0

                def _need(s):
                    nonlocal gi
                    while s not in mtss:
                        grp = groups[gi]
                        mt = _mbuild(grp)
                        for j, sx in enumerate(grp):
                            mtss[sx] = (mt, j * NB * 128)
                        gi += 1

                for s in range(NS):
                    T, rem4, pairs = geo[s]
                    _need(s)
                    if s + 1 < NS:
                        _need(s + 1)     # keep DVE a sentence ahead
                    mt, moff = mtss[s]
                    xt = xts[s]
                    for wtl in range(2):
                        wps = [(bi, kt) for bi, (kt, w) in enumerate(pairs)
                               if w == wtl]
                        pts = ppool.tile([128, 768], _F32, tag=f"p{wtl}",
                                         name="pts")
                        for i, (bi, kt) in enumerate(wps):
                            wk = rem4 if kt == FT else 128
                            st = (i == 0)
                            sp = (i == len(wps) - 1)
                            nc.tensor.matmul(
                                pts[:, :512],
                                mt[:wk, moff + bi * 128 : moff + bi * 128 + 128],
                                xt[:wk, kt * E : kt * E + 512],
                                start=st, stop=sp)
                            nc.tensor.matmul(
                                pts[:, 512:],
                                mt[:wk, moff + bi * 128 : moff + bi * 128 + 128],
                                xt[:wk, kt * E + 512 : (kt + 1) * E],
                                start=st, stop=sp)
                        col = 2 * s + wtl
                        ot = opool.tile([128, E], _BF16, tag="o", name="ot")
                        if s == NS - 1 and wtl == 1:
                            nc.vector.tensor_scalar_mul(
                                ot[:], pts[:], auxt[:, col : col + 1])
                        else:
                            nc.scalar.mul(
                                ot[:], pts[:], auxt[:, col : col + 1])
                        nc.sync.dma_start(
                            out=y[s * 256 + wtl * 128
                                  : s * 256 + wtl * 128 + 128, :],
                            in_=ot[:])
    nc.compile()
    return nc


def _xtof(entry):
    """xts[] entry -> (tile, col offset) for merged group tiles."""
    if isinstance(entry, tuple):
        return entry
    return entry, 0


def _make_in_maps_jm(output, mappings):
    """Host prep for mode "jm": sort sentences by total BPE rows, deal
    groups of 8 across cores (slot s, core k <- sentence order[s*8+k]);
    upload bf16 x in slot order, pre-replicated fp16 boundary rows,
    fp16 row-index columns, f32 1/m scales."""
    import ml_dtypes

    NS = BPC
    SW = 257
    output = np.asarray(output)
    mappings = np.asarray(mappings, dtype=np.int32)
    ends = np.cumsum(mappings, axis=1, dtype=np.int32)      # [B, W]
    Ti = ends[:, -1]                                        # [B]
    order = np.argsort(Ti, kind="stable")
    assign = order.reshape(NS, NCORES)                      # [slot, core]
    tmaxs = tuple(int(Ti[assign[s]].max()) for s in range(NS))
    xbf = output.astype(ml_dtypes.bfloat16)
    gcth = np.ascontiguousarray(
        (np.arange(128, dtype=np.float32)[:, None]
         + 128.0 * np.arange(4, dtype=np.float32)[None, :]).astype(np.float16))
    minv = 1.0 / mappings.astype(np.float32)
    in_maps = []
    for k in range(NCORES):
        sents = assign[:, k]
        xk = np.ascontiguousarray(xbf[sents].reshape(ROWS, E))
        sthk = np.zeros((NS, SW), np.float16)
        sthk[:, 1:256] = ends[sents]
        sthk[:, 256] = Ti[sents]
        sthr = np.ascontiguousarray(
            np.broadcast_to(sthk.reshape(1, -1), (128, NS * SW)))
        auxk = np.ones((128, NS * 2), np.float32)
        mi = minv[sents]
        for s in range(NS):
            auxk[:, 2 * s] = mi[s, 0:128]
            auxk[:127, 2 * s + 1] = mi[s, 128:255]
        in_maps.append({
            "x": xk,
            "sthr": sthr,
            "gcth": gcth,
            "aux": np.ascontiguousarray(auxk),
        })
    return in_maps, assign, tmaxs


def _run_jm(output, mappings, reps=1, warmups=7, pbufs=2, obufs=6,
            mbufs=4, **kw):
    in_maps, assign, tmaxs = _make_in_maps_jm(output, mappings)
    key = ("jm3", tmaxs, reps, warmups, pbufs, obufs, mbufs)
    if key not in _NC:
        _NC[key] = _build_nc_jm(tmaxs, reps, warmups, pbufs, obufs, mbufs)
    res = run_bass_kernel_spmd(_NC[key], in_maps, list(range(NCORES)), **kw)
    full = np.empty((B, W, E), np.float32)
    for k, r in enumerate(res.results):
        yk = np.asarray(r["y"], dtype=np.float32).reshape(BPC, 256, E)[:, :W]
        full[assign[:, k]] = yk
    return full, res


_NC = {}


def _get_nc(reps=1, bufs=2, order="pc", nq=1, mode="ab", nchunk=NCHUNK,
            merged_idx=False, warm=False, hscale=False, sizes=None,
            dt="f32", split0=False):
    key = (reps, bufs, order, nq, mode, nchunk, merged_idx, warm, hscale,
           tuple(sizes) if sizes else None, dt, split0)
    if key not in _NC:
        _NC[key] = _build_nc(reps, bufs, order, nq, mode, nchunk, merged_idx,
                             warm, hscale, sizes, dt, split0)
    return _NC[key]


def _wrap16(flat):
    """int16 index list -> [128, n/16] wrapped (i -> [i%16, i//16]) + 8x rep."""
    return np.tile(flat.reshape(-1, 16).T, (8, 1)).astype(np.int16)


def _make_in_maps(output, mappings, order="pc", mode="ab", nchunk=NCHUNK,
                  hscale=False, sizes=None, dt="f32"):
    output = np.asarray(output)
    if hscale:
        # fold the *0.5 of the span mean into the shard upload: a/2 + b/2
        # rounds identically to (a+b)/2 in f32 (halving is exact).
        output = output * np.float32(0.5)
    npdt = np.float32 if dt == "f32" else _np_bf16()
    mappings = np.asarray(mappings, dtype=np.int32)
    ends = np.cumsum(mappings, axis=1, dtype=np.int32)  # [B, W] exclusive ends
    src_a = ends - mappings + 1                         # +1: skip [CLS]
    src_b = ends                                        # (e-1) + 1
    if mode in ("wh", "wind"):
        # scale every BPE row by 1/m of its owning word (0.5/1 exact in f32,
        # single bf16 rounding after); junk rows keep scale 1 and are killed
        # on-device by beta=0.
        g = np.ones((B, S), np.float32)
        two = mappings == 2
        np.put_along_axis(
            g, src_a, np.where(two, np.float32(0.5), np.float32(1.0)), axis=1
        )
        bi, wi_ = np.nonzero(two)
        g[bi, src_a[bi, wi_] + 1] = 0.5
        output = output * g[:, :, None]
    output = np.ascontiguousarray(output.astype(npdt))

    if mode == "mm":
        in_maps = []
        gcol = (np.arange(128, dtype=np.float32)[:, None]
                + 128.0 * np.arange(4, dtype=np.float32)[None, :])
        gcol = np.ascontiguousarray(gcol)
        minv = 1.0 / mappings.astype(np.float32)          # [B, W]
        for k in range(NCORES):
            bs = slice(k * BPC, (k + 1) * BPC)
            sa = src_a[bs]                                # [8, 255] s-coords
            sth = np.empty((BPC, 257), np.float16)
            sth[:, :255] = sa
            sth[:, 255] = ends[bs, -1] + 1                # pad word start
            sth[:, 256] = 600.0                           # sentinel > 511
            sth = np.ascontiguousarray(sth.reshape(1, -1))  # [1, 2056]
            msc = np.ones((128, BPC * 2), np.float32)
            mi = minv[bs]                                 # [8, 255]
            for s_ in range(BPC):
                msc[:, s_ * 2] = mi[s_, 0:128]
                msc[:128 - 1, s_ * 2 + 1] = mi[s_, 128:255]
            xk = np.ascontiguousarray(
                output[bs].reshape(ROWS, E).astype(_np_bf16()))
            in_maps.append({"x": xk, "sth": sth, "gcol": gcol,
                            "msc": np.ascontiguousarray(msc)})
        return in_maps

    in_maps = []
    for k in range(NCORES):
        bs = slice(k * BPC, (k + 1) * BPC)
        base = (np.arange(BPC, dtype=np.int32) * S)[:, None]
        a = (src_a[bs] + base).reshape(-1)
        b = (src_b[bs] + base).reshape(-1)
        pad = np.zeros(NWP - NW, np.int32)
        a = np.concatenate([a, pad])  # [NWP] word-ordered flat row ids
        b = np.concatenate([b, pad])
        x = np.ascontiguousarray(output[bs].reshape(ROWS, E))
        if mode == "wind":
            wpp = NWP // nchunk // 128
            mm = np.concatenate(
                [mappings[bs].reshape(-1), np.ones(NWP - NW, np.int32)]
            )
            beta = (mm == 2).astype(npdt)
            # [p, q*wpp+j] = element offset of word q*cpw + p*wpp + j
            aw = a.reshape(nchunk, 128, wpp).transpose(1, 0, 2)  # [p, q, j]
            idx = np.ascontiguousarray(aw.reshape(128, -1).astype(np.int32))
            bt = np.empty((nchunk, 128, wpp), npdt)
            cpw_ = NWP // nchunk
            for q in range(nchunk):
                sl = slice(q * cpw_, (q + 1) * cpw_)
                bt[q] = beta[sl].reshape(128, wpp)
            in_maps.append({"x": x, "idx": idx, "bt": bt})
            continue
        if mode in ("ind", "acc"):
            nck = nchunk
            wpp = NWP // nck // 128
            ia = a.reshape(nck, 128, wpp).transpose(1, 0, 2)  # [p, q, j]
            ib = b.reshape(nck, 128, wpp).transpose(1, 0, 2)
            idx = np.concatenate(
                [np.concatenate([ia[:, q], ib[:, q]], axis=1)
                 for q in range(nck)],
                axis=1,
            ).astype(np.int32)  # [128, 2*wpp*nck], cols 2*wpp*q + j
            in_maps.append({"x": x, "idx": np.ascontiguousarray(idx)})
            continue
        if sizes is not None:
            segs = []
            off = 0
            for s in sizes:
                jj = s // 128
                aq = a[off : off + s].reshape(128, jj).T.ravel()
                bq = b[off : off + s].reshape(128, jj).T.ravel()
                segs.append(_wrap16(np.concatenate([aq, bq])))
                off += s
            in_maps.append({"x": x, "idx": np.concatenate(segs, axis=1)})
            continue
        cpw = NWP // nchunk
        jj = cpw // 128
        if mode == "ab":
            idx = np.empty((nchunk, 128, 2 * cpw // 16), np.int16)
            for q in range(nchunk):
                aq = a[q * cpw : (q + 1) * cpw]
                bq = b[q * cpw : (q + 1) * cpw]
                if order == "pc":
                    # gathered i = c*128 + p holds word q*cpw + p*jj + c
                    aq = aq.reshape(128, jj).T.ravel()
                    bq = bq.reshape(128, jj).T.ravel()
                # 'seq': gathered i holds word q*cpw + i (ascending rows)
                idx[q] = _wrap16(np.concatenate([aq, bq]))
            in_maps.append({"x": x, "idx": idx})
        elif mode == "wh":
            mm = np.concatenate(
                [mappings[bs].reshape(-1), np.ones(NWP - NW, np.int32)]
            )
            beta = (mm == 2).astype(npdt)
            idx = np.empty((nchunk, 128, cpw // 16), np.int16)
            bt = np.empty((nchunk, 128, jj), npdt)
            for q in range(nchunk):
                sl = slice(q * cpw, (q + 1) * cpw)
                idx[q] = _wrap16(a[sl].reshape(128, jj).T.ravel())
                bt[q] = beta[sl].reshape(128, jj)
            in_maps.append({"x": x, "idx": idx, "bt": bt})
        else:
            m = np.concatenate(
                [mappings[bs].reshape(-1), np.ones(NWP - NW, np.int32)]
            ).astype(np.float32)
            r1 = 1.0 / m
            r2 = (m - 1.0) / m
            rdt = np.float32 if mode == "ws" else npdt
            idx = np.empty((nchunk, 128, cpw // 16), np.int16)
            rw = np.empty((nchunk, 128, 2 * jj), rdt)
            for q in range(nchunk):
                sl = slice(q * cpw, (q + 1) * cpw)
                aq = a[sl].reshape(128, jj).T.ravel()  # i = c*128 + p
                idx[q] = _wrap16(aq)
                rw[q, :, 0::2] = r1[sl].reshape(128, jj).astype(rdt)
                rw[q, :, 1::2] = r2[sl].reshape(128, jj).astype(rdt)
            in_maps.append({"x": x, "idx": idx, "rw": rw})
    return in_maps


def _run(output, mappings, reps=1, bufs=2, order="pc", nq=1, mode="ab",
         nchunk=NCHUNK, merged_idx=False, warm=False, hscale=False,
         sizes=None, dt="f32", split0=False, warmups=7, pbufs=2,
         obufs=6, mbufs=4, **kw):
    if mode == "jm":
        return _run_jm(output, mappings, reps=reps, warmups=warmups,
                       pbufs=pbufs, obufs=obufs, mbufs=mbufs, **kw)
    in_maps = _make_in_maps(output, mappings, order, mode, nchunk, hscale,
                            sizes, dt)
    nc = _get_nc(reps, bufs, order, nq, mode, nchunk, merged_idx, warm,
                 hscale, sizes, dt, split0)
    res = run_bass_kernel_spmd(nc, in_maps, list(range(NCORES)), **kw)
    if mode == "mm":
        outs = [
            np.asarray(r["y"], dtype=np.float32).reshape(BPC, 256, E)[:, :W]
            for r in res.results
        ]
    else:
        outs = [
            np.asarray(r["y"][:NW], dtype=np.float32).reshape(BPC, W, E)
            for r in res.results
        ]
    return np.concatenate(outs, axis=0), res


# Best HW-verified configuration: JIT-specialized matmul segment-sum
# (mode "jm").  Prior best: mode "mm" @ ~42.4us median; runner-up kept
# working: dict(bufs=6, order="pc", nq=1, mode="wh", nchunk=8,
# merged_idx=True, split0=True, dt="bf16") @ ~52.7us.
_CFG = dict(mode="jm")


def kernel(output, mappings):
    full, _ = _run(output, mappings, **_CFG)
    return full

